# revision 3
# baseline (speedup 1.0000x reference)
"""Causal self-attention (B=2, T=2048, C=1024, H=16, D=64) on 8 TRN2 NeuronCores.

Sharding (Megatron-style, per the hint): data-parallel over the batch (B=2)
and tensor-parallel over heads (16 heads -> 4 groups of 4). Core c handles
batch b = c // 4 and head group g = c % 4:
  - qkv:    computes x[b] @ w_attn[:, cols-of-its-4-heads]  (column split)
  - attn:   full causal attention for its 4 heads
  - proj:   y_heads @ w_proj[rows-of-its-4-heads]           (row split)
The 4 partial proj outputs per batch are summed on the host (+ b_proj).

Device layout notes:
  - All matmuls run in bf16 (inputs pre-cast/pre-transposed on host), fp32
    PSUM accumulation.
  - Scores are computed transposed: S'[s, t] = (k_s . q_t)/8, so softmax sums
    over s (the partition dim) come for free out of the AV matmul by
    augmenting V with a ones column:  yT_aug = [V | 1]^T @ exp(S').
    Row 64 of yT_aug is the softmax denominator per t.
  - exp has no max-subtraction: logits are O(1) for this input distribution
    (|logit| < ~10), so fp32/bf16 exp is safe and the normalization cancels.

Scheduling: the PE p-state drops 2x after any idle gap, and the attention
S'->exp->AV chain alone cannot keep PE busy (ACT exp is ~1us/unit vs ~0.85us
of PE work). So qkv and proj matmul groups are woven INTO the attention
stream as filler work at statically chosen points, keeping the in-order PE
queue saturated: total PE work (~130us at full speed) becomes the makespan
instead of ACT-paced stalls at half PE clock.
"""

import os
import sys

sys.path.insert(0, "/opt/trn_rl_repo")

import numpy as np
import ml_dtypes

BF16 = ml_dtypes.bfloat16

B, T, C, H, D = 2, 2048, 1024, 16, 64
NCORES = 8
HG = 4          # heads per core
DQ = HG * D     # 256 qkv cols per core
CCH = C // 128  # 8 contraction chunks
NT = T // 128   # 16 token chunks of 128
NJ = T // 512   # 4 token tiles of 512

_NC_CACHE = {}


def build_nc(mm_dtype_name="bfloat16", loop=0, phases=("qkv", "attn", "proj"),
             attn_mode="full"):
    """loop=0: straight-line (graded path). loop=K>0: wrap the body in a
    device-side For_i repeat-K loop (timing builds only). phases: subset for
    bisection timing builds."""
    import contextlib
    import concourse.bacc as bacc
    import concourse.tile as tile
    from concourse import mybir

    mm_dt = getattr(mybir.dt, mm_dtype_name)
    f32 = mybir.dt.float32

    nc = bacc.Bacc("TRN2", target_bir_lowering=False, debug=False,
                   num_devices=NCORES)

    xT = nc.dram_tensor("xT", [C, T], mm_dt, kind="ExternalInput")
    wq = nc.dram_tensor("wq", [C, DQ], mm_dt, kind="ExternalInput")
    wk = nc.dram_tensor("wk", [C, DQ], mm_dt, kind="ExternalInput")
    wv = nc.dram_tensor("wv", [C, DQ], mm_dt, kind="ExternalInput")
    wp = nc.dram_tensor("wp", [DQ, C], mm_dt, kind="ExternalInput")
    bqk = nc.dram_tensor("bqk", [2, 2, 128], f32, kind="ExternalInput")  # [q/k, chunk, col]
    bv = nc.dram_tensor("bv", [128, DQ], f32, kind="ExternalInput")      # replicated
    mask = nc.dram_tensor("mask", [128, 128 + 4 * 512], mm_dt, kind="ExternalInput")
    out = nc.dram_tensor("out", [T, C], mm_dt, kind="ExternalOutput")

    with tile.TileContext(nc) as tc:
        with (
            tc.tile_pool(name="const", bufs=1) as const,
            tc.tile_pool(name="acts", bufs=1) as acts,
            tc.tile_pool(name="work", bufs=4) as work,
            tc.tile_pool(name="ostage", bufs=3) as ostage,
            tc.tile_pool(name="psum", bufs=1, space="PSUM") as psum,
            tc.tile_pool(name="psums", bufs=1, space="PSUM") as psums,
            tc.For_i(0, loop, 1,
                     hint_engines=(mybir.EngineType.PE,
                                   mybir.EngineType.Activation,
                                   mybir.EngineType.DVE,
                                   mybir.EngineType.SP,
                                   mybir.EngineType.Pool))
            if loop else contextlib.nullcontext(),
        ):
            # ---- constants / weights ----
            bqk_sb = const.tile([128, 2, 2, 1], f32)  # [col, q/k, chunk, 1]
            nc.sync.dma_start(out=bqk_sb, in_=bqk.rearrange("a m p -> p a m")[:, :, :, None])
            bv_sb = const.tile([128, DQ], f32)
            nc.sync.dma_start(out=bv_sb, in_=bv[:, :])
            # mask holds [ident(128) | 4 x 512 wide additive diag-block masks]
            maskc_sb = const.tile([128, 128], mm_dt)
            nc.sync.dma_start(out=maskc_sb, in_=mask[:, 0:128])
            maskw_sb = const.tile([128, 4, 512], mm_dt)
            nc.sync.dma_start(out=maskw_sb,
                              in_=mask[:, 128:].rearrange("p (a n) -> p a n", a=4))
            wk_sb = const.tile([128, CCH, DQ], mm_dt)
            nc.sync.dma_start(out=wk_sb, in_=wk.rearrange("(c p) m -> p c m", p=128))
            wq_sb = const.tile([128, CCH, DQ], mm_dt)
            nc.sync.dma_start(out=wq_sb, in_=wq.rearrange("(c p) m -> p c m", p=128))
            # xT loaded in 4 column windows so window-0 qkv starts early
            xT_sb = const.tile([128, CCH, T], mm_dt)
            xTr = xT.rearrange("(c p) t -> p c t", p=128)
            for jw in range(NJ):
                nc.sync.dma_start(out=xT_sb[:, :, 512 * jw:512 * jw + 512],
                                  in_=xTr[:, :, 512 * jw:512 * jw + 512])
            wv_sb = const.tile([128, CCH, DQ], mm_dt)
            nc.sync.dma_start(out=wv_sb, in_=wv.rearrange("(c p) m -> p c m", p=128))
            wp_sb = const.tile([128, 2, C], mm_dt)
            nc.sync.dma_start(out=wp_sb, in_=wp.rearrange("(k p) n -> p k n", p=128))

            # ---- activations ----
            qd_sb = acts.tile([128, 2, T], mm_dt)   # [dcol, chunk, t]
            kd_sb = acts.tile([128, 2, T], mm_dt)
            v_sb = acts.tile([128, NT, HG * 65], mm_dt)  # per s-chunk: 4x [V_h | 1]
            yt_sb = acts.tile([128, 2, T], mm_dt)

            # ones columns of v_sb (col 64 of each head slot)
            ones_view = v_sb.rearrange("p s (h e) -> p s h e", e=65)[:, :, :, 64:65]
            nc.vector.memset(ones_view, 1.0)

            # bisection timing builds: initialize tensors a skipped phase
            # would have produced
            if "qkv" not in phases:
                nc.vector.memset(qd_sb, 0.5)
                nc.vector.memset(kd_sb, 0.5)
                nc.vector.memset(v_sb, 0.5)
            if "attn" not in phases or attn_mode == "noav":
                nc.vector.memset(yt_sb, 0.5)

            # ---- PE work groups (attention units + filler groups) ----
            # Filler groups: one qk projection window, one V token-chunk, or
            # one proj token-chunk. Each allocates its psum from the shared
            # tag-"s" ring so PSUM stays within 8 banks.
            def qk_group(dst, wsb, qki, m, j):
                ps = psums.tile([128, 1024], f32, tag="s", bufs=3, name="ps_f")
                for c in range(CCH):
                    nc.tensor.matmul(
                        ps[:, 0:512],
                        lhsT=wsb[:, c, 128 * m:128 * m + 128],
                        rhs=xT_sb[:, c, 512 * j:512 * j + 512],
                        start=(c == 0), stop=(c == CCH - 1),
                    )
                nc.vector.tensor_scalar_add(
                    dst[:, m, 512 * j:512 * j + 512], ps[:, 0:512],
                    bqk_sb[:, qki, m, :],
                )

            def v_group(tt):
                # V in s-major [t, vcol]; out tile = xT_chunk(t)^T @ Wv_chunk
                ps = psums.tile([128, 1024], f32, tag="s", bufs=3, name="ps_f")
                for c in range(CCH):
                    nc.tensor.matmul(
                        ps[:, 0:DQ],
                        lhsT=xT_sb[:, c, 128 * tt:128 * tt + 128],
                        rhs=wv_sb[:, c, :],
                        start=(c == 0), stop=(c == CCH - 1),
                    )
                nc.vector.tensor_tensor(
                    v_sb.rearrange("p s (h e) -> p s h e", e=65)[:, tt, :, 0:64],
                    ps[:, 0:DQ].rearrange("p (h d) -> p h d", d=64),
                    bv_sb.rearrange("p (h d) -> p h d", d=64),
                    mybir.AluOpType.add,
                )

            def proj_group(tt):
                ps = psums.tile([128, 1024], f32, tag="s", bufs=3, name="ps_o")
                for n2 in range(2):
                    for kc in range(2):
                        nc.tensor.matmul(
                            ps[:, 512 * n2:512 * n2 + 512],
                            lhsT=yt_sb[:, kc, 128 * tt:128 * tt + 128],
                            rhs=wp_sb[:, kc, 512 * n2:512 * n2 + 512],
                            start=(kc == 0), stop=(kc == 1),
                        )
                os_sb = ostage.tile([128, C], mm_dt, tag="osb", name="os_sb")
                # Pool/GPSIMD cannot read PSUM on TRN2; DVE does the drain
                nc.vector.tensor_copy(os_sb, ps)
                nc.sync.dma_start(out=out[128 * tt:128 * tt + 128, :],
                                  in_=os_sb)

            # ---- attention units, software-pipelined (see module docstring)
            exp_f = mybir.ActivationFunctionType.Exp
            LAG = int(os.environ.get("ATTN_LAG", "2"))

            pending = []  # queue of emitted-S'/exp units awaiting AV emission

            def flush_unit():
                u = pending.pop(0)
                for mmargs in u["av"]:
                    nc.tensor.matmul(**mmargs)
                if u["fin"] is not None:
                    h, j, yps = u["fin"]
                    m, roff = divmod(h, 2)
                    roff *= 64
                    r = work.tile([1, 512], f32, tag="r", bufs=2, name="r")
                    nc.vector.reciprocal(r, yps[64:65, :])
                    rr = work.tile([64, 512], f32, tag="rr", bufs=2, name="rr")
                    nc.gpsimd.partition_broadcast(rr, r)
                    nc.vector.tensor_tensor(
                        yt_sb[roff:roff + 64, m, 512 * j:512 * j + 512],
                        yps[0:64, :], rr, mybir.AluOpType.mult,
                    )

            def attn_unit(h, j, i2, yps):
                """Emit S' matmuls + exp for unit i2 of head-window (h, j)."""
                m, roff = divmod(h, 2)
                roff *= 64
                kd_h = kd_sb[roff:roff + 64, m, :]
                qd_h = qd_sb[roff:roff + 64, m, :]
                jwin = slice(512 * j, 512 * (j + 1))
                nI = 4 * j + 4
                sps = psums.tile([128, 1024], f32, tag="s", bufs=3, name="sps")
                nomask = "nomask" in attn_mode
                for u in (0, 1):
                    i = 2 * i2 + u
                    d = i - 4 * j  # >= 0 for diagonal-block chunks
                    nc.tensor.matmul(
                        sps[:, 512 * u:512 * u + 512],
                        lhsT=kd_h[:, 128 * i:128 * i + 128],
                        rhs=qd_h[:, jwin],
                        start=True, stop=(d < 0 or nomask),
                    )
                    if d >= 0 and not nomask:
                        nc.tensor.matmul(
                            sps[:, 512 * u:512 * u + 128 * (d + 1)],
                            lhsT=maskc_sb,                   # identity
                            rhs=maskw_sb[:, d, 0:128 * (d + 1)],
                            start=False, stop=True,
                        )
                pt = work.tile([128, 1024], mm_dt, tag="p", bufs=6, name="pt")
                f = exp_f if "expcopy" not in attn_mode else \
                    mybir.ActivationFunctionType.Copy
                nc.scalar.activation(pt, sps, f, scale=0.125)
                if attn_mode == "noav":
                    return
                av = [dict(out=yps[0:65, :],
                           lhsT=v_sb[:, 2 * i2 + u, 65 * h:65 * h + 65],
                           rhs=pt[:, 512 * u:512 * u + 512],
                           start=(2 * i2 + u == 0),
                           stop=(2 * i2 + u == nI - 1))
                      for u in (0, 1)]
                pending.append(dict(
                    av=av, fin=(h, j, yps) if i2 == 2 * j + 1 else None))
                while len(pending) > LAG:
                    flush_unit()

            # ---- static filler schedule ----
            # Keyed by the attention unit (j, h, i2) after which each filler
            # group is emitted. Ordering constraints:
            #   - qk m-group window w emitted before any S' of (heads of m, w)
            #   - V chunk tt emitted before AV of chunk tt (LAG units later)
            #   - proj tt of window w emitted after window w's last normalize
            #     (which flushes LAG units into window w+1)
            QF = "qkv" in phases
            PF = "proj" in phases and attn_mode != "noav"
            K_, Q_, V_, P_ = "K", "Q", "V", "P"
            sched_list = [
                ((0, 0, 0), [(V_, 0), (V_, 1)]),
                ((0, 0, 1), [(V_, 2), (V_, 3), (K_, 1, 0)]),
                ((0, 1, 0), [(Q_, 1, 0)]),
                ((0, 1, 1), [(K_, 0, 1)]),
                ((0, 2, 0), [(Q_, 0, 1)]),
                ((0, 2, 1), [(V_, 4)]),
                ((0, 3, 0), [(V_, 5), (K_, 1, 1)]),
                ((0, 3, 1), [(Q_, 1, 1)]),
                ((1, 0, 0), [(V_, 6)]),
                ((1, 0, 1), [(V_, 7)]),
                ((1, 0, 2), [(P_, 0)]),
                ((1, 0, 3), [(P_, 1)]),
                ((1, 1, 0), [(P_, 2)]),
                ((1, 1, 1), [(P_, 3)]),
                ((1, 1, 2), [(K_, 0, 2)]),
                ((1, 1, 3), [(Q_, 0, 2)]),
                ((1, 2, 0), [(V_, 8)]),
                ((1, 2, 2), [(V_, 9)]),
                ((1, 3, 0), [(K_, 1, 2)]),
                ((1, 3, 2), [(Q_, 1, 2)]),
                ((2, 0, 0), [(V_, 10)]),
                ((2, 0, 2), [(V_, 11)]),
                ((2, 0, 4), [(P_, 4)]),
                ((2, 1, 0), [(P_, 5)]),
                ((2, 1, 2), [(P_, 6)]),
                ((2, 1, 4), [(P_, 7)]),
                ((2, 2, 0), [(K_, 0, 3)]),
                ((2, 2, 2), [(Q_, 0, 3)]),
                ((2, 2, 4), [(V_, 12)]),
                ((2, 3, 0), [(V_, 13)]),
                ((2, 3, 2), [(K_, 1, 3)]),
                ((2, 3, 4), [(Q_, 1, 3)]),
                ((3, 0, 0), [(V_, 14)]),
                ((3, 0, 2), [(V_, 15)]),
                ((3, 0, 4), [(P_, 8)]),
                ((3, 0, 6), [(P_, 9)]),
                ((3, 1, 0), [(P_, 10)]),
                ((3, 1, 2), [(P_, 11)]),
            ]
            tail_fillers = [(P_, tt) for tt in range(12, 16)]

            def run_filler(f):
                kind = f[0]
                if kind == K_ and QF:
                    qk_group(kd_sb, wk_sb, 1, f[1], f[2])
                elif kind == Q_ and QF:
                    qk_group(qd_sb, wq_sb, 0, f[1], f[2])
                elif kind == V_ and QF:
                    v_group(f[1])
                elif kind == P_ and PF:
                    proj_group(f[1])

            sched = {}
            for key, fl in sched_list:
                sched.setdefault(key, []).extend(fl)

            if "attn" in phases:
                # preamble: just enough qkv for (h=0, j=0) to start
                if QF:
                    qk_group(kd_sb, wk_sb, 1, 0, 0)
                    qk_group(qd_sb, wq_sb, 0, 0, 0)
                hlist = [0, 2, 0, 2] if "evenheads" in attn_mode else range(HG)
                for j in range(NJ):
                    for h in hlist:
                        yps = None
                        if attn_mode != "noav":
                            yps = psum.tile([128, 512], f32, tag="y", bufs=2,
                                            name="yps")
                        for i2 in range(2 * j + 2):
                            attn_unit(h, j, i2, yps)
                            for f in sched.get((j, h, i2), ()):
                                run_filler(f)
                while pending:
                    flush_unit()
                for f in tail_fillers:
                    run_filler(f)
            else:
                # no attention: run the groups sequentially for bisection
                if QF:
                    qk_group(kd_sb, wk_sb, 1, 0, 0)
                    qk_group(qd_sb, wq_sb, 0, 0, 0)
                for _, fl in sched_list:
                    for f in fl:
                        run_filler(f)
                for f in tail_fillers:
                    run_filler(f)

    nc.finalize()
    return nc


def make_in_maps(x, w_attn, b_attn, w_proj):
    x = np.asarray(x, dtype=np.float32)
    w_attn = np.asarray(w_attn, dtype=np.float32)
    b_attn = np.asarray(b_attn, dtype=np.float32)
    w_proj = np.asarray(w_proj, dtype=np.float32)

    ident = np.eye(128, dtype=np.float32)
    tri = np.where(np.triu(np.ones((128, 128), bool)), 0.0, -3000.0)
    blocks = [ident]
    for d in range(4):
        blk = np.zeros((128, 512), np.float32)
        blk[:, :128 * d] = -3000.0
        blk[:, 128 * d:128 * d + 128] = tri
        blocks.append(blk)
    mask_np = np.concatenate(blocks, axis=1).astype(BF16)  # [128, 128+2048]
    xts = [np.ascontiguousarray(x[b].T).astype(BF16) for b in range(B)]
    in_maps = []
    for core in range(NCORES):
        b, g = divmod(core, 4)
        cq = slice(0 * C + g * DQ, 0 * C + (g + 1) * DQ)
        ck = slice(1 * C + g * DQ, 1 * C + (g + 1) * DQ)
        cv = slice(2 * C + g * DQ, 2 * C + (g + 1) * DQ)
        bq = b_attn[cq]
        bk = b_attn[ck]
        bqk = np.stack([bq.reshape(2, 128), bk.reshape(2, 128)]).astype(np.float32)
        bv = np.broadcast_to(b_attn[cv], (128, DQ)).copy().astype(np.float32)
        in_maps.append({
            "xT": xts[b],
            "wq": np.ascontiguousarray(w_attn[:, cq]).astype(BF16),
            "wk": np.ascontiguousarray(w_attn[:, ck]).astype(BF16),
            "wv": np.ascontiguousarray(w_attn[:, cv]).astype(BF16),
            "wp": np.ascontiguousarray(w_proj[g * DQ:(g + 1) * DQ, :]).astype(BF16),
            "bqk": bqk,
            "bv": bv,
            "mask": mask_np,
        })
    return in_maps


def kernel(x, w_attn, b_attn, w_proj, b_proj):
    from concourse.bass_utils import run_bass_kernel_spmd

    if "nc" not in _NC_CACHE:
        _NC_CACHE["nc"] = build_nc()
    nc = _NC_CACHE["nc"]

    in_maps = make_in_maps(x, w_attn, b_attn, w_proj)
    res = run_bass_kernel_spmd(nc, in_maps, list(range(NCORES)))

    b_proj = np.asarray(b_proj, dtype=np.float32)
    out = np.zeros((B, T, C), np.float32)
    for core in range(NCORES):
        b = core // 4
        out[b] += res.results[core]["out"].astype(np.float32)
    out += b_proj[None, None, :]
    return out


# revision 7
# speedup vs baseline: 1.2994x; 1.2994x over previous
"""Causal self-attention (B=2, T=2048, C=1024, H=16, D=64) on 8 TRN2 NeuronCores.

Sharding (Megatron-style, per the hint): data-parallel over the batch (B=2)
and tensor-parallel over heads (16 heads -> 4 groups of 4). Core c handles
batch b = c // 4 and head group g = c % 4:
  - qkv:    computes x[b] @ w_attn[:, cols-of-its-4-heads]  (column split)
  - attn:   full causal attention for its 4 heads
  - proj:   y_heads @ w_proj[rows-of-its-4-heads]           (row split)
The 4 partial proj outputs per batch are summed on the host (+ b_proj).

Device layout notes:
  - All matmuls run in bf16 (inputs pre-cast/pre-transposed on host), fp32
    PSUM accumulation. The device output is bf16 (upcast on host).
  - Scores are computed transposed: S'[s, t] = (k_s . q_t)/8, so softmax sums
    over s (the partition dim) come for free out of the AV matmul by
    augmenting V with a ones column:  yT_aug = [V | 1]^T @ exp(S').
    Row 64 of yT_aug is the softmax denominator per t.
  - exp has no max-subtraction: logits are O(1) for this input distribution
    (|logit| < ~10), so fp32/bf16 exp is safe and the normalization cancels.

Scheduling notes (measured on HW, see exp_micro*.py):
  - Alternating the PE between 64- and 128-partition stationaries costs
    ~440ns per switch. K is therefore stored zero-PADDED per head
    (kd_pad[:, h]: head h's 64 d-rows at their natural partitions, zeros
    elsewhere) so S' runs as a uniform 128-partition matmul; the zero rows
    null out the other head's q rows in the full-width rhs.
  - The attention S'->exp->AV chain leaves PE idle gaps (ACT exp ~1.2us per
    [128,1024] tile), so qkv and proj matmul groups are woven INTO the
    attention stream as filler work at statically chosen points.
  - Diagonal-block chunks are narrowed to their causally-valid column range
    (plus a 128-wide triangular boundary mask folded into the PE
    accumulation group via an identity matmul).
"""

import os
import sys

sys.path.insert(0, "/opt/trn_rl_repo")

import numpy as np
import ml_dtypes

BF16 = ml_dtypes.bfloat16

B, T, C, H, D = 2, 2048, 1024, 16, 64
NCORES = 8
HG = 4          # heads per core
DQ = HG * D     # 256 qkv cols per core
CCH = C // 128  # 8 contraction chunks
NT = T // 128   # 16 token chunks of 128
NJ = T // 512   # 4 token tiles of 512

_NC_CACHE = {}


def build_nc(mm_dtype_name="bfloat16", loop=0, phases=("qkv", "attn", "proj"),
             attn_mode="full", narrow=True):
    """loop=0: straight-line (graded path). loop=K>0: wrap the body in a
    device-side For_i repeat-K loop (timing builds only). phases: subset for
    bisection timing builds."""
    import contextlib
    import concourse.bacc as bacc
    import concourse.tile as tile
    from concourse import mybir

    mm_dt = getattr(mybir.dt, mm_dtype_name)
    f32 = mybir.dt.float32

    nc = bacc.Bacc("TRN2", target_bir_lowering=False, debug=False,
                   num_devices=NCORES)

    xT = nc.dram_tensor("xT", [C, T], mm_dt, kind="ExternalInput")
    wq = nc.dram_tensor("wq", [C, DQ], mm_dt, kind="ExternalInput")
    wk = nc.dram_tensor("wk", [C, DQ], mm_dt, kind="ExternalInput")
    wv = nc.dram_tensor("wv", [C, DQ], mm_dt, kind="ExternalInput")
    wp = nc.dram_tensor("wp", [DQ, C], mm_dt, kind="ExternalInput")
    bqk = nc.dram_tensor("bqk", [2, 2, 128], f32, kind="ExternalInput")  # [q/k, chunk, col]
    bv = nc.dram_tensor("bv", [128, DQ], f32, kind="ExternalInput")      # replicated
    mask = nc.dram_tensor("mask", [128, 128 + 4 * 512], mm_dt, kind="ExternalInput")
    out = nc.dram_tensor("out", [T, C], mm_dt, kind="ExternalOutput")

    with tile.TileContext(nc) as tc:
        with (
            tc.tile_pool(name="const", bufs=1) as const,
            tc.tile_pool(name="acts", bufs=1) as acts,
            tc.tile_pool(name="work", bufs=4) as work,
            tc.tile_pool(name="ostage", bufs=3) as ostage,
            tc.tile_pool(name="psum", bufs=1, space="PSUM") as psum,
            tc.tile_pool(name="psums", bufs=1, space="PSUM") as psums,
        ):
            # ---- activations (allocated once; persistent across loop) ----
            qd_sb = acts.tile([128, 2, T], mm_dt)   # [dcol, chunk, t]
            kd_pad = acts.tile([128, HG, T], mm_dt)  # per-head, zero-padded
            v_sb = acts.tile([128, NT, HG * 65], mm_dt)  # per s-chunk: 4x [V_h | 1]
            yt_sb = acts.tile([128, 2, T], mm_dt)

            # one-time init (never overwritten by the loop body): the zero
            # padding rows of kd_pad and the ones columns of v_sb
            nc.vector.memset(kd_pad, 0.0)
            ones_view = v_sb.rearrange("p s (h e) -> p s h e", e=65)[:, :, :, 64:65]
            nc.vector.memset(ones_view, 1.0)
            # bisection timing builds: initialize tensors a skipped phase
            # would have produced
            if "qkv" not in phases:
                nc.vector.memset(qd_sb, 0.5)
                for h in range(HG):
                    roff = 64 * (h % 2)
                    nc.vector.memset(kd_pad[roff:roff + 64, h, :], 0.5)
                nc.vector.memset(
                    v_sb.rearrange("p s (h e) -> p s h e", e=65)[:, :, :, 0:64],
                    0.5)
            if "attn" not in phases or attn_mode == "noav":
                nc.vector.memset(yt_sb, 0.5)

            with (
                tc.For_i(0, loop, 1,
                         hint_engines=(mybir.EngineType.PE,
                                       mybir.EngineType.Activation,
                                       mybir.EngineType.DVE,
                                       mybir.EngineType.SP,
                                       mybir.EngineType.Pool))
                if loop else contextlib.nullcontext()
            ):
                # ---- constants / weights ----
                bqk_sb = const.tile([128, 2, 2, 1], f32)  # [col, q/k, chunk, 1]
                nc.sync.dma_start(out=bqk_sb,
                                  in_=bqk.rearrange("a m p -> p a m")[:, :, :, None])
                bv_sb = const.tile([128, DQ], f32)
                nc.sync.dma_start(out=bv_sb, in_=bv[:, :])
                # mask holds [ident(128) | 4 x 512 wide additive diag-block masks]
                maskc_sb = const.tile([128, 128], mm_dt)
                nc.sync.dma_start(out=maskc_sb, in_=mask[:, 0:128])
                maskw_sb = const.tile([128, 4, 512], mm_dt)
                nc.sync.dma_start(out=maskw_sb,
                                  in_=mask[:, 128:].rearrange("p (a n) -> p a n", a=4))
                wk_sb = const.tile([128, CCH, DQ], mm_dt)
                nc.sync.dma_start(out=wk_sb, in_=wk.rearrange("(c p) m -> p c m", p=128))
                wq_sb = const.tile([128, CCH, DQ], mm_dt)
                nc.sync.dma_start(out=wq_sb, in_=wq.rearrange("(c p) m -> p c m", p=128))
                # xT loaded in 4 column windows so window-0 qkv starts early
                xT_sb = const.tile([128, CCH, T], mm_dt)
                xTr = xT.rearrange("(c p) t -> p c t", p=128)
                for jw in range(NJ):
                    nc.sync.dma_start(out=xT_sb[:, :, 512 * jw:512 * jw + 512],
                                      in_=xTr[:, :, 512 * jw:512 * jw + 512])
                wv_sb = const.tile([128, CCH, DQ], mm_dt)
                nc.sync.dma_start(out=wv_sb, in_=wv.rearrange("(c p) m -> p c m", p=128))
                wp_sb = const.tile([128, 2, C], mm_dt)
                nc.sync.dma_start(out=wp_sb, in_=wp.rearrange("(k p) n -> p k n", p=128))

                # ---- PE work groups (attention units + filler groups) ----
                # Filler groups allocate psum from the shared tag-"s" ring so
                # PSUM stays within 8 banks.
                def qk_group(qki, m, j):
                    ps = psums.tile([128, 1024], f32, tag="s", bufs=3, name="ps_f")
                    wsb = wq_sb if qki == 0 else wk_sb
                    for c in range(CCH):
                        nc.tensor.matmul(
                            ps[:, 0:512],
                            lhsT=wsb[:, c, 128 * m:128 * m + 128],
                            rhs=xT_sb[:, c, 512 * j:512 * j + 512],
                            start=(c == 0), stop=(c == CCH - 1),
                        )
                    jw = slice(512 * j, 512 * j + 512)
                    if qki == 0:
                        nc.vector.tensor_scalar_add(
                            qd_sb[:, m, jw], ps[:, 0:512], bqk_sb[:, 0, m, :])
                    else:
                        # split into the two per-head zero-padded slots
                        nc.vector.tensor_scalar_add(
                            kd_pad[0:64, 2 * m, jw], ps[0:64, 0:512],
                            bqk_sb[0:64, 1, m, :])
                        nc.vector.tensor_scalar_add(
                            kd_pad[64:128, 2 * m + 1, jw], ps[64:128, 0:512],
                            bqk_sb[64:128, 1, m, :])

                def v_group(tt):
                    # V in s-major [t, vcol]; out tile = xT_chunk(t)^T @ Wv_chunk
                    ps = psums.tile([128, 1024], f32, tag="s", bufs=3, name="ps_f")
                    for c in range(CCH):
                        nc.tensor.matmul(
                            ps[:, 0:DQ],
                            lhsT=xT_sb[:, c, 128 * tt:128 * tt + 128],
                            rhs=wv_sb[:, c, :],
                            start=(c == 0), stop=(c == CCH - 1),
                        )
                    nc.vector.tensor_tensor(
                        v_sb.rearrange("p s (h e) -> p s h e", e=65)[:, tt, :, 0:64],
                        ps[:, 0:DQ].rearrange("p (h d) -> p h d", d=64),
                        bv_sb.rearrange("p (h d) -> p h d", d=64),
                        mybir.AluOpType.add,
                    )

                def proj_group(tt):
                    ps = psums.tile([128, 1024], f32, tag="s", bufs=3, name="ps_o")
                    for n2 in range(2):
                        for kc in range(2):
                            nc.tensor.matmul(
                                ps[:, 512 * n2:512 * n2 + 512],
                                lhsT=yt_sb[:, kc, 128 * tt:128 * tt + 128],
                                rhs=wp_sb[:, kc, 512 * n2:512 * n2 + 512],
                                start=(kc == 0), stop=(kc == 1),
                            )
                    os_sb = ostage.tile([128, C], mm_dt, tag="osb", name="os_sb")
                    nc.vector.tensor_copy(os_sb, ps)
                    nc.sync.dma_start(out=out[128 * tt:128 * tt + 128, :],
                                      in_=os_sb)

                # ---- attention units, software-pipelined ----
                exp_f = mybir.ActivationFunctionType.Exp
                LAG = int(os.environ.get("ATTN_LAG", "2"))

                pending = []  # emitted-S'/exp units awaiting AV emission

                def flush_unit():
                    u = pending.pop(0)
                    for mmargs in u["av"]:
                        nc.tensor.matmul(**mmargs)
                    if u["fin"] is not None:
                        h, j, yps = u["fin"]
                        m, roff = divmod(h, 2)
                        roff *= 64
                        r = work.tile([1, 512], f32, tag="r", bufs=2, name="r")
                        nc.vector.reciprocal(r, yps[64:65, :])
                        rr = work.tile([64, 512], f32, tag="rr", bufs=2, name="rr")
                        nc.gpsimd.partition_broadcast(rr, r)
                        nc.vector.tensor_tensor(
                            yt_sb[roff:roff + 64, m, 512 * j:512 * j + 512],
                            yps[0:64, :], rr, mybir.AluOpType.mult,
                        )

                def attn_unit(h, j, i2, yps):
                    """Emit S' matmuls + exp for unit i2 of head-window (h, j)."""
                    m = h // 2
                    kd_h = kd_pad[:, h, :]
                    qd_h = qd_sb[:, m, :]
                    jwin = slice(512 * j, 512 * (j + 1))
                    nI = 4 * j + 4
                    sps = psums.tile([128, 1024], f32, tag="s", bufs=3, name="sps")
                    nomask = "nomask" in attn_mode
                    for u in (0, 1):
                        i = 2 * i2 + u
                        d = i - 4 * j  # >= 0 for diagonal-block chunks
                        if narrow and d > 0 and not nomask:
                            # diagonal chunk: columns < 128d are fully masked;
                            # compute only [128d, 512) plus a 128-wide
                            # triangular boundary mask (maskw block 0 is tri)
                            off = 128 * d
                            nc.tensor.matmul(
                                sps[:, 512 * u + off:512 * u + 512],
                                lhsT=kd_h[:, 128 * i:128 * i + 128],
                                rhs=qd_h[:, 512 * j + off:512 * j + 512],
                                start=True, stop=False,
                            )
                            nc.tensor.matmul(
                                sps[:, 512 * u + off:512 * u + off + 128],
                                lhsT=maskc_sb,                   # identity
                                rhs=maskw_sb[:, 0, 0:128],       # tri block
                                start=False, stop=True,
                            )
                            continue
                        nc.tensor.matmul(
                            sps[:, 512 * u:512 * u + 512],
                            lhsT=kd_h[:, 128 * i:128 * i + 128],
                            rhs=qd_h[:, jwin],
                            start=True, stop=(d < 0 or nomask),
                        )
                        if d >= 0 and not nomask:
                            nc.tensor.matmul(
                                sps[:, 512 * u:512 * u + 128 * (d + 1)],
                                lhsT=maskc_sb,                   # identity
                                rhs=maskw_sb[:, d, 0:128 * (d + 1)],
                                start=False, stop=True,
                            )
                    pt = work.tile([128, 1024], mm_dt, tag="p", bufs=6, name="pt")
                    f = exp_f if "expcopy" not in attn_mode else \
                        mybir.ActivationFunctionType.Copy
                    nc.scalar.activation(pt, sps, f, scale=0.125)
                    if attn_mode == "noav":
                        return
                    av = []
                    for u in (0, 1):
                        i = 2 * i2 + u
                        d = i - 4 * j
                        off = 128 * d if (narrow and d > 0) else 0
                        av.append(dict(out=yps[0:65, off:512],
                                       lhsT=v_sb[:, i, 65 * h:65 * h + 65],
                                       rhs=pt[:, 512 * u + off:512 * u + 512],
                                       start=(i == 0),
                                       stop=(i == nI - 1)))
                    pending.append(dict(
                        av=av, fin=(h, j, yps) if i2 == 2 * j + 1 else None))
                    while len(pending) > LAG:
                        flush_unit()

                # ---- static filler schedule ----
                # Keyed by the attention unit (j, h, i2) after which each
                # filler group is emitted. Ordering constraints:
                #   - qk m-group window w emitted before any S' of (heads of
                #     m, w)
                #   - V chunk tt emitted before AV of chunk tt
                #   - proj tt of window w emitted after window w's last
                #     normalize (which flushes LAG units into window w+1)
                QF = "qkv" in phases
                PF = "proj" in phases and attn_mode != "noav"
                K_, Q_, V_, P_ = "K", "Q", "V", "P"
                sched_list = [
                    ((0, 0, 0), [(V_, 0), (V_, 1)]),
                    ((0, 0, 1), [(V_, 2), (V_, 3), (K_, 1, 0)]),
                    ((0, 1, 0), [(Q_, 1, 0)]),
                    ((0, 1, 1), [(K_, 0, 1)]),
                    ((0, 2, 0), [(Q_, 0, 1)]),
                    ((0, 2, 1), [(V_, 4)]),
                    ((0, 3, 0), [(V_, 5), (K_, 1, 1)]),
                    ((0, 3, 1), [(Q_, 1, 1)]),
                    ((1, 0, 0), [(V_, 6)]),
                    ((1, 0, 1), [(V_, 7)]),
                    ((1, 0, 2), [(P_, 0)]),
                    ((1, 0, 3), [(P_, 1)]),
                    ((1, 1, 0), [(P_, 2)]),
                    ((1, 1, 1), [(P_, 3)]),
                    ((1, 1, 2), [(K_, 0, 2)]),
                    ((1, 1, 3), [(Q_, 0, 2)]),
                    ((1, 2, 0), [(V_, 8)]),
                    ((1, 2, 2), [(V_, 9)]),
                    ((1, 3, 0), [(K_, 1, 2)]),
                    ((1, 3, 2), [(Q_, 1, 2)]),
                    ((2, 0, 0), [(V_, 10)]),
                    ((2, 0, 2), [(V_, 11)]),
                    ((2, 0, 4), [(P_, 4)]),
                    ((2, 1, 0), [(P_, 5)]),
                    ((2, 1, 2), [(P_, 6)]),
                    ((2, 1, 4), [(P_, 7)]),
                    ((2, 2, 0), [(K_, 0, 3)]),
                    ((2, 2, 2), [(Q_, 0, 3)]),
                    ((2, 2, 4), [(V_, 12)]),
                    ((2, 3, 0), [(V_, 13)]),
                    ((2, 3, 2), [(K_, 1, 3)]),
                    ((2, 3, 4), [(Q_, 1, 3)]),
                    ((3, 0, 0), [(V_, 14)]),
                    ((3, 0, 2), [(V_, 15)]),
                    ((3, 0, 4), [(P_, 8)]),
                    ((3, 0, 6), [(P_, 9)]),
                    ((3, 1, 0), [(P_, 10)]),
                    ((3, 1, 2), [(P_, 11)]),
                ]
                tail_fillers = [(P_, tt) for tt in range(12, 16)]

                def run_filler(f):
                    kind = f[0]
                    if kind == K_ and QF:
                        qk_group(1, f[1], f[2])
                    elif kind == Q_ and QF:
                        qk_group(0, f[1], f[2])
                    elif kind == V_ and QF:
                        v_group(f[1])
                    elif kind == P_ and PF:
                        proj_group(f[1])

                sched = {}
                for key, fl in sched_list:
                    sched.setdefault(key, []).extend(fl)

                if "attn" in phases:
                    # preamble: just enough qkv for (h=0, j=0) to start
                    if QF:
                        qk_group(1, 0, 0)
                        qk_group(0, 0, 0)
                    hlist = [0, 2, 0, 2] if "evenheads" in attn_mode else range(HG)
                    for j in range(NJ):
                        for h in hlist:
                            yps = None
                            if attn_mode != "noav":
                                yps = psum.tile([128, 512], f32, tag="y",
                                                bufs=2, name="yps")
                            for i2 in range(2 * j + 2):
                                attn_unit(h, j, i2, yps)
                                for f in sched.get((j, h, i2), ()):
                                    run_filler(f)
                    while pending:
                        flush_unit()
                    for f in tail_fillers:
                        run_filler(f)
                else:
                    # no attention: run the groups sequentially for bisection
                    if QF:
                        qk_group(1, 0, 0)
                        qk_group(0, 0, 0)
                    for _, fl in sched_list:
                        for f in fl:
                            run_filler(f)
                    for f in tail_fillers:
                        run_filler(f)

    nc.finalize()
    return nc


def make_in_maps(x, w_attn, b_attn, w_proj):
    x = np.asarray(x, dtype=np.float32)
    w_attn = np.asarray(w_attn, dtype=np.float32)
    b_attn = np.asarray(b_attn, dtype=np.float32)
    w_proj = np.asarray(w_proj, dtype=np.float32)

    ident = np.eye(128, dtype=np.float32)
    tri = np.where(np.triu(np.ones((128, 128), bool)), 0.0, -3000.0)
    blocks = [ident]
    for d in range(4):
        blk = np.zeros((128, 512), np.float32)
        blk[:, :128 * d] = -3000.0
        blk[:, 128 * d:128 * d + 128] = tri
        blocks.append(blk)
    mask_np = np.concatenate(blocks, axis=1).astype(BF16)  # [128, 128+2048]
    xts = [np.ascontiguousarray(x[b].T).astype(BF16) for b in range(B)]
    in_maps = []
    for core in range(NCORES):
        b, g = divmod(core, 4)
        cq = slice(0 * C + g * DQ, 0 * C + (g + 1) * DQ)
        ck = slice(1 * C + g * DQ, 1 * C + (g + 1) * DQ)
        cv = slice(2 * C + g * DQ, 2 * C + (g + 1) * DQ)
        bq = b_attn[cq]
        bk = b_attn[ck]
        bqk = np.stack([bq.reshape(2, 128), bk.reshape(2, 128)]).astype(np.float32)
        bv = np.broadcast_to(b_attn[cv], (128, DQ)).copy().astype(np.float32)
        in_maps.append({
            "xT": xts[b],
            "wq": np.ascontiguousarray(w_attn[:, cq]).astype(BF16),
            "wk": np.ascontiguousarray(w_attn[:, ck]).astype(BF16),
            "wv": np.ascontiguousarray(w_attn[:, cv]).astype(BF16),
            "wp": np.ascontiguousarray(w_proj[g * DQ:(g + 1) * DQ, :]).astype(BF16),
            "bqk": bqk,
            "bv": bv,
            "mask": mask_np,
        })
    return in_maps


def kernel(x, w_attn, b_attn, w_proj, b_proj):
    from concourse.bass_utils import run_bass_kernel_spmd

    if "nc" not in _NC_CACHE:
        _NC_CACHE["nc"] = build_nc()
    nc = _NC_CACHE["nc"]

    in_maps = make_in_maps(x, w_attn, b_attn, w_proj)
    res = run_bass_kernel_spmd(nc, in_maps, list(range(NCORES)))

    b_proj = np.asarray(b_proj, dtype=np.float32)
    out = np.zeros((B, T, C), np.float32)
    for core in range(NCORES):
        b = core // 4
        out[b] += res.results[core]["out"].astype(np.float32)
    out += b_proj[None, None, :]
    return out


# revision 11
# speedup vs baseline: 1.4126x; 1.0872x over previous
"""Causal self-attention (B=2, T=2048, C=1024, H=16, D=64) on 8 TRN2 NeuronCores.

Sharding (Megatron-style, per the hint): data-parallel over the batch (B=2)
and tensor-parallel over heads (16 heads -> 4 groups of 4). Core c handles
batch b = c // 4 and head group g = c % 4:
  - qkv:    computes x[b] @ w_attn[:, cols-of-its-4-heads]  (column split)
  - attn:   full causal attention for its 4 heads
  - proj:   y_heads @ w_proj[rows-of-its-4-heads]           (row split)
The 4 partial proj outputs per batch are summed on the host (+ b_proj).

Device layout notes:
  - All matmuls run in bf16 (inputs pre-cast/pre-transposed on host), fp32
    PSUM accumulation. The device output is bf16 (upcast on host).
  - Scores are computed transposed: S'[s, t] = (k_s . q_t)/8, so softmax sums
    over s (the partition dim) come for free out of the AV matmul by
    augmenting V with a ones column:  yT_aug = [V | 1]^T @ exp(S').
    Row 64 of yT_aug is the softmax denominator per t.
  - exp has no max-subtraction: logits are O(1) for this input distribution
    (|logit| < ~10), so fp32/bf16 exp is safe and the normalization cancels.

Scheduling notes (measured on HW, see exp_micro*.py):
  - Alternating the PE between 64- and 128-partition stationaries costs
    ~440ns per switch. K is therefore stored zero-PADDED per head
    (kd_pad[:, h]: head h's 64 d-rows at their natural partitions, zeros
    elsewhere) so S' runs as a uniform 128-partition matmul; the zero rows
    null out the other head's q rows in the full-width rhs.
  - The attention S'->exp->AV chain leaves PE idle gaps (ACT exp ~1.2us per
    [128,1024] tile), so qkv and proj matmul groups are woven INTO the
    attention stream as filler work at statically chosen points.
  - Diagonal-block chunks are narrowed to their causally-valid column range
    (plus a 128-wide triangular boundary mask folded into the PE
    accumulation group via an identity matmul).
"""

import os
import sys

sys.path.insert(0, "/opt/trn_rl_repo")

import numpy as np
import ml_dtypes

BF16 = ml_dtypes.bfloat16

B, T, C, H, D = 2, 2048, 1024, 16, 64
NCORES = 8
HG = 4          # heads per core
DQ = HG * D     # 256 qkv cols per core
CCH = C // 128  # 8 contraction chunks
NT = T // 128   # 16 token chunks of 128
NJ = T // 512   # 4 token tiles of 512

_NC_CACHE = {}


def build_nc(mm_dtype_name="bfloat16", loop=0, phases=("qkv", "attn", "proj"),
             attn_mode="full", narrow=True):
    """loop=0: straight-line (graded path). loop=K>0: wrap the body in a
    device-side For_i repeat-K loop (timing builds only). phases: subset for
    bisection timing builds."""
    import contextlib
    import concourse.bacc as bacc
    import concourse.tile as tile
    from concourse import mybir

    mm_dt = getattr(mybir.dt, mm_dtype_name)
    f32 = mybir.dt.float32

    nc = bacc.Bacc("TRN2", target_bir_lowering=False, debug=False,
                   num_devices=NCORES)

    xT = nc.dram_tensor("xT", [C, T], mm_dt, kind="ExternalInput")
    wq = nc.dram_tensor("wq", [C, DQ], mm_dt, kind="ExternalInput")
    wk = nc.dram_tensor("wk", [C, DQ], mm_dt, kind="ExternalInput")
    wv = nc.dram_tensor("wv", [C, DQ], mm_dt, kind="ExternalInput")
    wp = nc.dram_tensor("wp", [DQ, C], mm_dt, kind="ExternalInput")
    bqk = nc.dram_tensor("bqk", [2, 2, 128], f32, kind="ExternalInput")  # [q/k, chunk, col]
    bv = nc.dram_tensor("bv", [128, DQ], f32, kind="ExternalInput")      # replicated
    mask = nc.dram_tensor("mask", [128, 128 + 4 * 512], mm_dt, kind="ExternalInput")
    out = nc.dram_tensor("out", [T, C], mm_dt, kind="ExternalOutput")

    with tile.TileContext(nc) as tc:
        with (
            tc.tile_pool(name="const", bufs=1) as const,
            tc.tile_pool(name="acts", bufs=1) as acts,
            tc.tile_pool(name="work", bufs=4) as work,
            tc.tile_pool(name="ostage", bufs=3) as ostage,
            tc.tile_pool(name="psum", bufs=1, space="PSUM") as psum,
            tc.tile_pool(name="psums", bufs=1, space="PSUM") as psums,
        ):
            # ---- activations (allocated once; persistent across loop) ----
            qd_sb = acts.tile([128, 2, T], mm_dt)   # [dcol, chunk, t]
            kd_pad = acts.tile([128, HG, T], mm_dt)  # per-head, zero-padded
            v_sb = acts.tile([128, NT, HG * 65], mm_dt)  # per s-chunk: 4x [V_h | 1]
            yt_sb = acts.tile([128, 2, T], mm_dt)

            # one-time init (never overwritten by the loop body): the zero
            # padding rows of kd_pad and the ones columns of v_sb
            nc.vector.memset(kd_pad, 0.0)
            ones_view = v_sb.rearrange("p s (h e) -> p s h e", e=65)[:, :, :, 64:65]
            nc.vector.memset(ones_view, 1.0)
            # bisection timing builds: initialize tensors a skipped phase
            # would have produced
            if "qkv" not in phases:
                nc.vector.memset(qd_sb, 0.5)
                for h in range(HG):
                    roff = 64 * (h % 2)
                    nc.vector.memset(kd_pad[roff:roff + 64, h, :], 0.5)
                nc.vector.memset(
                    v_sb.rearrange("p s (h e) -> p s h e", e=65)[:, :, :, 0:64],
                    0.5)
            if "attn" not in phases or attn_mode == "noav":
                nc.vector.memset(yt_sb, 0.5)

            with (
                tc.For_i(0, loop, 1,
                         hint_engines=(mybir.EngineType.PE,
                                       mybir.EngineType.Activation,
                                       mybir.EngineType.DVE,
                                       mybir.EngineType.SP,
                                       mybir.EngineType.Pool))
                if loop else contextlib.nullcontext()
            ):
                # ---- constants / weights ----
                bqk_sb = const.tile([128, 2, 2, 1], f32)  # [col, q/k, chunk, 1]
                nc.sync.dma_start(out=bqk_sb,
                                  in_=bqk.rearrange("a m p -> p a m")[:, :, :, None])
                bv_sb = const.tile([128, DQ], f32)
                nc.sync.dma_start(out=bv_sb, in_=bv[:, :])
                # mask holds [ident(128) | 4 x 512 wide additive diag-block masks]
                maskc_sb = const.tile([128, 128], mm_dt)
                nc.sync.dma_start(out=maskc_sb, in_=mask[:, 0:128])
                maskw_sb = const.tile([128, 4, 512], mm_dt)
                nc.sync.dma_start(out=maskw_sb,
                                  in_=mask[:, 128:].rearrange("p (a n) -> p a n", a=4))
                wk_sb = const.tile([128, CCH, DQ], mm_dt)
                nc.sync.dma_start(out=wk_sb, in_=wk.rearrange("(c p) m -> p c m", p=128))
                wq_sb = const.tile([128, CCH, DQ], mm_dt)
                nc.sync.dma_start(out=wq_sb, in_=wq.rearrange("(c p) m -> p c m", p=128))
                # xT loaded in 4 column windows so window-0 qkv starts early
                xT_sb = const.tile([128, CCH, T], mm_dt)
                xTr = xT.rearrange("(c p) t -> p c t", p=128)
                for jw in range(NJ):
                    nc.sync.dma_start(out=xT_sb[:, :, 512 * jw:512 * jw + 512],
                                      in_=xTr[:, :, 512 * jw:512 * jw + 512])
                wv_sb = const.tile([128, CCH, DQ], mm_dt)
                nc.sync.dma_start(out=wv_sb, in_=wv.rearrange("(c p) m -> p c m", p=128))
                wp_sb = const.tile([128, 2, C], mm_dt)
                nc.sync.dma_start(out=wp_sb, in_=wp.rearrange("(k p) n -> p k n", p=128))

                # ---- PE work groups (attention units + filler groups) ----
                # Filler groups allocate psum from the shared tag-"s" ring so
                # PSUM stays within 8 banks.
                def qk_group(qki, m, j):
                    ps = psums.tile([128, 1024], f32, tag="s", bufs=3, name="ps_f")
                    wsb = wq_sb if qki == 0 else wk_sb
                    for c in range(CCH):
                        nc.tensor.matmul(
                            ps[:, 0:512],
                            lhsT=wsb[:, c, 128 * m:128 * m + 128],
                            rhs=xT_sb[:, c, 512 * j:512 * j + 512],
                            start=(c == 0), stop=(c == CCH - 1),
                        )
                    jw = slice(512 * j, 512 * j + 512)
                    if qki == 0:
                        nc.vector.tensor_scalar_add(
                            qd_sb[:, m, jw], ps[:, 0:512], bqk_sb[:, 0, m, :])
                    else:
                        # split into the two per-head zero-padded slots
                        nc.vector.tensor_scalar_add(
                            kd_pad[0:64, 2 * m, jw], ps[0:64, 0:512],
                            bqk_sb[0:64, 1, m, :])
                        nc.vector.tensor_scalar_add(
                            kd_pad[64:128, 2 * m + 1, jw], ps[64:128, 0:512],
                            bqk_sb[64:128, 1, m, :])

                def v_group(tt):
                    # V in s-major [t, vcol]; out tile = xT_chunk(t)^T @ Wv_chunk
                    ps = psums.tile([128, 1024], f32, tag="s", bufs=3, name="ps_f")
                    for c in range(CCH):
                        nc.tensor.matmul(
                            ps[:, 0:DQ],
                            lhsT=xT_sb[:, c, 128 * tt:128 * tt + 128],
                            rhs=wv_sb[:, c, :],
                            start=(c == 0), stop=(c == CCH - 1),
                        )
                    nc.vector.tensor_tensor(
                        v_sb.rearrange("p s (h e) -> p s h e", e=65)[:, tt, :, 0:64],
                        ps[:, 0:DQ].rearrange("p (h d) -> p h d", d=64),
                        bv_sb.rearrange("p (h d) -> p h d", d=64),
                        mybir.AluOpType.add,
                    )

                def proj_group(tt):
                    ps = psums.tile([128, 1024], f32, tag="s", bufs=3, name="ps_o")
                    for n2 in range(2):
                        for kc in range(2):
                            nc.tensor.matmul(
                                ps[:, 512 * n2:512 * n2 + 512],
                                lhsT=yt_sb[:, kc, 128 * tt:128 * tt + 128],
                                rhs=wp_sb[:, kc, 512 * n2:512 * n2 + 512],
                                start=(kc == 0), stop=(kc == 1),
                            )
                    os_sb = ostage.tile([128, C], mm_dt, tag="osb", name="os_sb")
                    # drain on ACT: keeps the shared psum ring's release off
                    # the (busier) DVE queue
                    nc.scalar.copy(os_sb, ps)
                    nc.sync.dma_start(out=out[128 * tt:128 * tt + 128, :],
                                      in_=os_sb)

                # ---- attention units, software-pipelined ----
                exp_f = mybir.ActivationFunctionType.Exp
                LAG = int(os.environ.get("ATTN_LAG", "2"))

                pending = []  # emitted-S'/exp units awaiting AV emission

                def flush_unit():
                    u = pending.pop(0)
                    for mmargs in u["av"]:
                        nc.tensor.matmul(**mmargs)
                    if u["fin"] is not None:
                        h, j, yps = u["fin"]
                        m, roff = divmod(h, 2)
                        roff *= 64
                        r = work.tile([1, 512], f32, tag="r", bufs=2, name="r")
                        nc.vector.reciprocal(r, yps[64:65, :])
                        rr = work.tile([64, 512], f32, tag="rr", bufs=2, name="rr")
                        nc.gpsimd.partition_broadcast(rr, r)
                        nc.vector.tensor_tensor(
                            yt_sb[roff:roff + 64, m, 512 * j:512 * j + 512],
                            yps[0:64, :], rr, mybir.AluOpType.mult,
                        )

                def attn_unit(h, j, i2, yps):
                    """Emit S' matmuls + exp for unit i2 of head-window (h, j)."""
                    m = h // 2
                    kd_h = kd_pad[:, h, :]
                    qd_h = qd_sb[:, m, :]
                    jwin = slice(512 * j, 512 * (j + 1))
                    nI = 4 * j + 4
                    sps = psums.tile([128, 1024], f32, tag="s", bufs=3, name="sps")
                    nomask = "nomask" in attn_mode
                    for u in (0, 1):
                        i = 2 * i2 + u
                        d = i - 4 * j  # >= 0 for diagonal-block chunks
                        if narrow and d > 0 and not nomask:
                            # diagonal chunk: columns < 128d are fully masked;
                            # compute only [128d, 512) plus a 128-wide
                            # triangular boundary mask (maskw block 0 is tri)
                            off = 128 * d
                            nc.tensor.matmul(
                                sps[:, 512 * u + off:512 * u + 512],
                                lhsT=kd_h[:, 128 * i:128 * i + 128],
                                rhs=qd_h[:, 512 * j + off:512 * j + 512],
                                start=True, stop=False,
                            )
                            nc.tensor.matmul(
                                sps[:, 512 * u + off:512 * u + off + 128],
                                lhsT=maskc_sb,                   # identity
                                rhs=maskw_sb[:, 0, 0:128],       # tri block
                                start=False, stop=True,
                            )
                            continue
                        nc.tensor.matmul(
                            sps[:, 512 * u:512 * u + 512],
                            lhsT=kd_h[:, 128 * i:128 * i + 128],
                            rhs=qd_h[:, jwin],
                            start=True, stop=(d < 0 or nomask),
                        )
                        if d >= 0 and not nomask:
                            nc.tensor.matmul(
                                sps[:, 512 * u:512 * u + 128 * (d + 1)],
                                lhsT=maskc_sb,                   # identity
                                rhs=maskw_sb[:, d, 0:128 * (d + 1)],
                                start=False, stop=True,
                            )
                    pt = work.tile([128, 1024], mm_dt, tag="p", bufs=6, name="pt")
                    f = exp_f if "expcopy" not in attn_mode else \
                        mybir.ActivationFunctionType.Copy
                    nc.scalar.activation(pt, sps, f, scale=0.125)
                    if attn_mode == "noav":
                        return
                    av = []
                    for u in (0, 1):
                        i = 2 * i2 + u
                        d = i - 4 * j
                        off = 128 * d if (narrow and d > 0) else 0
                        av.append(dict(out=yps[0:65, off:512],
                                       lhsT=v_sb[:, i, 65 * h:65 * h + 65],
                                       rhs=pt[:, 512 * u + off:512 * u + 512],
                                       start=(i == 0),
                                       stop=(i == nI - 1)))
                    pending.append(dict(
                        av=av, fin=(h, j, yps) if i2 == 2 * j + 1 else None))
                    while len(pending) > LAG:
                        flush_unit()

                # ---- static filler schedule ----
                # Keyed by the attention unit (j, h, i2) after which each
                # filler group is emitted. Ordering constraints:
                #   - qk m-group window w emitted before any S' of (heads of
                #     m, w)
                #   - V chunk tt emitted before AV of chunk tt
                #   - proj tt of window w emitted after window w's last
                #     normalize (which flushes LAG units into window w+1)
                QF = "qkv" in phases
                PF = "proj" in phases and attn_mode != "noav"
                K_, Q_, V_, P_ = "K", "Q", "V", "P"
                # at most ONE filler per unit slot: a clump of fillers puts
                # multiple allocations between consecutive sps tiles of the
                # shared ring, making the S' pipeline wait on drain latency
                sched_list = [
                    ((0, 0, 0), [(V_, 2)]),
                    ((0, 0, 1), [(V_, 3)]),
                    ((0, 1, 0), [(K_, 1, 0)]),
                    ((0, 1, 1), [(Q_, 1, 0)]),
                    ((0, 2, 0), [(K_, 0, 1)]),
                    ((0, 2, 1), [(Q_, 0, 1)]),
                    ((0, 3, 0), [(V_, 4)]),
                    ((0, 3, 1), [(V_, 5)]),
                    ((1, 0, 0), [(V_, 6)]),
                    ((1, 0, 1), [(V_, 7)]),
                    ((1, 0, 2), [(P_, 0)]),
                    ((1, 0, 3), [(P_, 1)]),
                    ((1, 1, 0), [(P_, 2)]),
                    ((1, 1, 1), [(P_, 3)]),
                    ((1, 1, 2), [(K_, 1, 1)]),
                    ((1, 1, 3), [(Q_, 1, 1)]),
                    ((1, 2, 0), [(V_, 8)]),
                    ((1, 2, 1), [(V_, 9)]),
                    ((1, 2, 2), [(K_, 0, 2)]),
                    ((1, 2, 3), [(Q_, 0, 2)]),
                    ((1, 3, 0), [(V_, 10)]),
                    ((1, 3, 1), [(V_, 11)]),
                    ((1, 3, 2), [(K_, 1, 2)]),
                    ((1, 3, 3), [(Q_, 1, 2)]),
                    ((2, 0, 2), [(P_, 4)]),
                    ((2, 0, 4), [(P_, 5)]),
                    ((2, 1, 0), [(P_, 6)]),
                    ((2, 1, 2), [(P_, 7)]),
                    ((2, 1, 4), [(V_, 12)]),
                    ((2, 2, 0), [(V_, 13)]),
                    ((2, 2, 2), [(K_, 0, 3)]),
                    ((2, 2, 4), [(Q_, 0, 3)]),
                    ((2, 3, 0), [(V_, 14)]),
                    ((2, 3, 2), [(V_, 15)]),
                    ((2, 3, 4), [(K_, 1, 3)]),
                    ((3, 0, 0), [(Q_, 1, 3)]),
                    ((3, 0, 2), [(P_, 8)]),
                    ((3, 0, 4), [(P_, 9)]),
                    ((3, 0, 6), [(P_, 10)]),
                    ((3, 1, 0), [(P_, 11)]),
                ]
                tail_fillers = [(P_, tt) for tt in range(12, 16)]

                def run_filler(f):
                    kind = f[0]
                    if kind == K_ and QF:
                        qk_group(1, f[1], f[2])
                    elif kind == Q_ and QF:
                        qk_group(0, f[1], f[2])
                    elif kind == V_ and QF:
                        v_group(f[1])
                    elif kind == P_ and PF:
                        proj_group(f[1])

                sched = {}
                for key, fl in sched_list:
                    sched.setdefault(key, []).extend(fl)

                if "attn" in phases:
                    # preamble: just enough qkv for (h=0, j=0) to start
                    # (V chunks 0,1 are consumed by the first flushed AVs)
                    if QF:
                        qk_group(1, 0, 0)
                        qk_group(0, 0, 0)
                        v_group(0)
                        v_group(1)
                    hlist = [0, 2, 0, 2] if "evenheads" in attn_mode else range(HG)
                    for j in range(NJ):
                        for h in hlist:
                            yps = None
                            if attn_mode != "noav":
                                yps = psum.tile([128, 512], f32, tag="y",
                                                bufs=2, name="yps")
                            for i2 in range(2 * j + 2):
                                attn_unit(h, j, i2, yps)
                                for f in sched.get((j, h, i2), ()):
                                    run_filler(f)
                    while pending:
                        flush_unit()
                    for f in tail_fillers:
                        run_filler(f)
                else:
                    # no attention: run the groups sequentially for bisection
                    if QF:
                        qk_group(1, 0, 0)
                        qk_group(0, 0, 0)
                        v_group(0)
                        v_group(1)
                    for _, fl in sched_list:
                        for f in fl:
                            run_filler(f)
                    for f in tail_fillers:
                        run_filler(f)

    nc.finalize()
    return nc


def make_in_maps(x, w_attn, b_attn, w_proj):
    x = np.asarray(x, dtype=np.float32)
    w_attn = np.asarray(w_attn, dtype=np.float32)
    b_attn = np.asarray(b_attn, dtype=np.float32)
    w_proj = np.asarray(w_proj, dtype=np.float32)

    ident = np.eye(128, dtype=np.float32)
    tri = np.where(np.triu(np.ones((128, 128), bool)), 0.0, -3000.0)
    blocks = [ident]
    for d in range(4):
        blk = np.zeros((128, 512), np.float32)
        blk[:, :128 * d] = -3000.0
        blk[:, 128 * d:128 * d + 128] = tri
        blocks.append(blk)
    mask_np = np.concatenate(blocks, axis=1).astype(BF16)  # [128, 128+2048]
    xts = [np.ascontiguousarray(x[b].T).astype(BF16) for b in range(B)]
    in_maps = []
    for core in range(NCORES):
        b, g = divmod(core, 4)
        cq = slice(0 * C + g * DQ, 0 * C + (g + 1) * DQ)
        ck = slice(1 * C + g * DQ, 1 * C + (g + 1) * DQ)
        cv = slice(2 * C + g * DQ, 2 * C + (g + 1) * DQ)
        bq = b_attn[cq]
        bk = b_attn[ck]
        bqk = np.stack([bq.reshape(2, 128), bk.reshape(2, 128)]).astype(np.float32)
        bv = np.broadcast_to(b_attn[cv], (128, DQ)).copy().astype(np.float32)
        in_maps.append({
            "xT": xts[b],
            "wq": np.ascontiguousarray(w_attn[:, cq]).astype(BF16),
            "wk": np.ascontiguousarray(w_attn[:, ck]).astype(BF16),
            "wv": np.ascontiguousarray(w_attn[:, cv]).astype(BF16),
            "wp": np.ascontiguousarray(w_proj[g * DQ:(g + 1) * DQ, :]).astype(BF16),
            "bqk": bqk,
            "bv": bv,
            "mask": mask_np,
        })
    return in_maps


def kernel(x, w_attn, b_attn, w_proj, b_proj):
    from concourse.bass_utils import run_bass_kernel_spmd

    if "nc" not in _NC_CACHE:
        _NC_CACHE["nc"] = build_nc()
    nc = _NC_CACHE["nc"]

    in_maps = make_in_maps(x, w_attn, b_attn, w_proj)
    res = run_bass_kernel_spmd(nc, in_maps, list(range(NCORES)))

    b_proj = np.asarray(b_proj, dtype=np.float32)
    out = np.zeros((B, T, C), np.float32)
    for core in range(NCORES):
        b = core // 4
        out[b] += res.results[core]["out"].astype(np.float32)
    out += b_proj[None, None, :]
    return out


# revision 15
# speedup vs baseline: 1.4518x; 1.0277x over previous
"""Causal self-attention (B=2, T=2048, C=1024, H=16, D=64) on 8 TRN2 NeuronCores.

Sharding (Megatron-style, per the hint): data-parallel over the batch (B=2)
and tensor-parallel over heads (16 heads -> 4 groups of 4). Core c handles
batch b = c // 4 and head group g = c % 4:
  - qkv:    computes x[b] @ w_attn[:, cols-of-its-4-heads]  (column split)
  - attn:   full causal attention for its 4 heads
  - proj:   y_heads @ w_proj[rows-of-its-4-heads]           (row split)
The 4 partial proj outputs per batch are summed on the host (+ b_proj).

Device layout notes:
  - All matmuls run in bf16 (inputs pre-cast/pre-transposed on host), fp32
    PSUM accumulation. The device output is bf16 (upcast on host).
  - Scores are computed transposed: S'[s, t] = (k_s . q_t)/8, so softmax sums
    over s (the partition dim) come for free out of the AV matmul by
    augmenting V with a ones column:  yT_aug = [V | 1]^T @ exp(S').
    Row 64 of yT_aug is the softmax denominator per t.
  - exp has no max-subtraction: logits are O(1) for this input distribution
    (|logit| < ~10), so fp32/bf16 exp is safe and the normalization cancels.

Scheduling notes (measured on HW, see exp_micro*.py):
  - Alternating the PE between 64- and 128-partition stationaries costs
    ~440ns per switch. K is therefore stored zero-PADDED per head
    (kd_pad[:, h]: head h's 64 d-rows at their natural partitions, zeros
    elsewhere) so S' runs as a uniform 128-partition matmul; the zero rows
    null out the other head's q rows in the full-width rhs.
  - The attention S'->exp->AV chain leaves PE idle gaps (ACT exp ~1.2us per
    [128,1024] tile), so qkv and proj matmul groups are woven INTO the
    attention stream as filler work at statically chosen points.
  - Diagonal-block chunks are narrowed to their causally-valid column range
    (plus a 128-wide triangular boundary mask folded into the PE
    accumulation group via an identity matmul).
"""

import os
import sys

sys.path.insert(0, "/opt/trn_rl_repo")

import numpy as np
import ml_dtypes

BF16 = ml_dtypes.bfloat16

B, T, C, H, D = 2, 2048, 1024, 16, 64
NCORES = 8
HG = 4          # heads per core
DQ = HG * D     # 256 qkv cols per core
CCH = C // 128  # 8 contraction chunks
NT = T // 128   # 16 token chunks of 128
NJ = T // 512   # 4 token tiles of 512

_NC_CACHE = {}


def build_nc(mm_dtype_name="bfloat16", loop=0, phases=("qkv", "attn", "proj"),
             attn_mode="full", narrow=True):
    """loop=0: straight-line (graded path). loop=K>0: wrap the body in a
    device-side For_i repeat-K loop (timing builds only). phases: subset for
    bisection timing builds."""
    import contextlib
    import concourse.bacc as bacc
    import concourse.tile as tile
    from concourse import mybir

    mm_dt = getattr(mybir.dt, mm_dtype_name)
    f32 = mybir.dt.float32

    nc = bacc.Bacc("TRN2", target_bir_lowering=False, debug=False,
                   num_devices=NCORES)

    xT = nc.dram_tensor("xT", [C, T], mm_dt, kind="ExternalInput")
    wq = nc.dram_tensor("wq", [C, DQ], mm_dt, kind="ExternalInput")
    wk = nc.dram_tensor("wk", [C, DQ], mm_dt, kind="ExternalInput")
    wv = nc.dram_tensor("wv", [C, DQ], mm_dt, kind="ExternalInput")
    wp = nc.dram_tensor("wp", [DQ, C], mm_dt, kind="ExternalInput")
    bqk = nc.dram_tensor("bqk", [2, 2, 128], f32, kind="ExternalInput")  # [q/k, chunk, col]
    bv = nc.dram_tensor("bv", [128, DQ], f32, kind="ExternalInput")      # replicated
    mask = nc.dram_tensor("mask", [128, 128 + 4 * 512], mm_dt, kind="ExternalInput")
    out = nc.dram_tensor("out", [T, C], mm_dt, kind="ExternalOutput")

    with tile.TileContext(nc) as tc:
        with (
            tc.tile_pool(name="const", bufs=1) as const,
            tc.tile_pool(name="acts", bufs=1) as acts,
            tc.tile_pool(name="work", bufs=4) as work,
            tc.tile_pool(name="ostage", bufs=3) as ostage,
            tc.tile_pool(name="psum", bufs=1, space="PSUM") as psum,
            tc.tile_pool(name="psums", bufs=1, space="PSUM") as psums,
        ):
            # ---- activations (allocated once; persistent across loop) ----
            qd_sb = acts.tile([128, 2, T], mm_dt)   # [dcol, chunk, t]
            kd_pad = acts.tile([128, HG, T], mm_dt)  # per-head, zero-padded
            v_sb = acts.tile([128, NT, HG * 65], mm_dt)  # per s-chunk: 4x [V_h | 1]
            yt_sb = acts.tile([128, 2, T], mm_dt)

            # one-time init (never overwritten by the loop body): the zero
            # padding rows of kd_pad and the ones columns of v_sb
            nc.vector.memset(kd_pad, 0.0)
            ones_view = v_sb.rearrange("p s (h e) -> p s h e", e=65)[:, :, :, 64:65]
            nc.vector.memset(ones_view, 1.0)
            # bisection timing builds: initialize tensors a skipped phase
            # would have produced
            if "qkv" not in phases:
                nc.vector.memset(qd_sb, 0.5)
                for h in range(HG):
                    roff = 64 * (h % 2)
                    nc.vector.memset(kd_pad[roff:roff + 64, h, :], 0.5)
                nc.vector.memset(
                    v_sb.rearrange("p s (h e) -> p s h e", e=65)[:, :, :, 0:64],
                    0.5)
            if "attn" not in phases or attn_mode == "noav":
                nc.vector.memset(yt_sb, 0.5)

            with (
                tc.For_i(0, loop, 1,
                         hint_engines=(mybir.EngineType.PE,
                                       mybir.EngineType.Activation,
                                       mybir.EngineType.DVE,
                                       mybir.EngineType.SP,
                                       mybir.EngineType.Pool))
                if loop else contextlib.nullcontext()
            ):
                # ---- constants / weights ----
                bqk_sb = const.tile([128, 2, 2, 1], f32)  # [col, q/k, chunk, 1]
                nc.sync.dma_start(out=bqk_sb,
                                  in_=bqk.rearrange("a m p -> p a m")[:, :, :, None])
                bv_sb = const.tile([128, DQ], f32)
                nc.sync.dma_start(out=bv_sb, in_=bv[:, :])
                # mask holds [ident(128) | 4 x 512 wide additive diag-block masks]
                maskc_sb = const.tile([128, 128], mm_dt)
                nc.sync.dma_start(out=maskc_sb, in_=mask[:, 0:128])
                maskw_sb = const.tile([128, 4, 512], mm_dt)
                nc.sync.dma_start(out=maskw_sb,
                                  in_=mask[:, 128:].rearrange("p (a n) -> p a n", a=4))
                wk_sb = const.tile([128, CCH, DQ], mm_dt)
                nc.sync.dma_start(out=wk_sb, in_=wk.rearrange("(c p) m -> p c m", p=128))
                wq_sb = const.tile([128, CCH, DQ], mm_dt)
                nc.sync.dma_start(out=wq_sb, in_=wq.rearrange("(c p) m -> p c m", p=128))
                # xT loaded in 4 column windows so window-0 qkv starts early
                xT_sb = const.tile([128, CCH, T], mm_dt)
                xTr = xT.rearrange("(c p) t -> p c t", p=128)
                for jw in range(NJ):
                    nc.sync.dma_start(out=xT_sb[:, :, 512 * jw:512 * jw + 512],
                                      in_=xTr[:, :, 512 * jw:512 * jw + 512])
                wv_sb = const.tile([128, CCH, DQ], mm_dt)
                nc.sync.dma_start(out=wv_sb, in_=wv.rearrange("(c p) m -> p c m", p=128))
                wp_sb = const.tile([128, 2, C], mm_dt)
                nc.sync.dma_start(out=wp_sb, in_=wp.rearrange("(k p) n -> p k n", p=128))

                # ---- PE work groups (attention units + filler groups) ----
                # Filler groups allocate psum from the shared tag-"s" ring so
                # PSUM stays within 8 banks.
                def qk_group(qki, m, j):
                    ps = psums.tile([128, 1024], f32, tag="s", bufs=3, name="ps_f")
                    wsb = wq_sb if qki == 0 else wk_sb
                    for c in range(CCH):
                        nc.tensor.matmul(
                            ps[:, 0:512],
                            lhsT=wsb[:, c, 128 * m:128 * m + 128],
                            rhs=xT_sb[:, c, 512 * j:512 * j + 512],
                            start=(c == 0), stop=(c == CCH - 1),
                        )
                    jw = slice(512 * j, 512 * j + 512)
                    if qki == 0:
                        nc.vector.tensor_scalar_add(
                            qd_sb[:, m, jw], ps[:, 0:512], bqk_sb[:, 0, m, :])
                    else:
                        # split into the two per-head zero-padded slots
                        nc.vector.tensor_scalar_add(
                            kd_pad[0:64, 2 * m, jw], ps[0:64, 0:512],
                            bqk_sb[0:64, 1, m, :])
                        nc.vector.tensor_scalar_add(
                            kd_pad[64:128, 2 * m + 1, jw], ps[64:128, 0:512],
                            bqk_sb[64:128, 1, m, :])

                def v_group(tt):
                    # V in s-major [t, vcol]; out tile = xT_chunk(t)^T @ Wv_chunk
                    ps = psums.tile([128, 1024], f32, tag="s", bufs=3, name="ps_f")
                    for c in range(CCH):
                        nc.tensor.matmul(
                            ps[:, 0:DQ],
                            lhsT=xT_sb[:, c, 128 * tt:128 * tt + 128],
                            rhs=wv_sb[:, c, :],
                            start=(c == 0), stop=(c == CCH - 1),
                        )
                    nc.vector.tensor_tensor(
                        v_sb.rearrange("p s (h e) -> p s h e", e=65)[:, tt, :, 0:64],
                        ps[:, 0:DQ].rearrange("p (h d) -> p h d", d=64),
                        bv_sb.rearrange("p (h d) -> p h d", d=64),
                        mybir.AluOpType.add,
                    )

                def proj_group(tt):
                    ps = psums.tile([128, 1024], f32, tag="s", bufs=3, name="ps_o")
                    for n2 in range(2):
                        for kc in range(2):
                            nc.tensor.matmul(
                                ps[:, 512 * n2:512 * n2 + 512],
                                lhsT=yt_sb[:, kc, 128 * tt:128 * tt + 128],
                                rhs=wp_sb[:, kc, 512 * n2:512 * n2 + 512],
                                start=(kc == 0), stop=(kc == 1),
                            )
                    os_sb = ostage.tile([128, C], mm_dt, tag="osb", name="os_sb")
                    # drain on ACT: keeps the shared psum ring's release off
                    # the (busier) DVE queue
                    nc.scalar.copy(os_sb, ps)
                    nc.sync.dma_start(out=out[128 * tt:128 * tt + 128, :],
                                      in_=os_sb)

                # ---- attention units, software-pipelined ----
                exp_f = mybir.ActivationFunctionType.Exp
                LAG = int(os.environ.get("ATTN_LAG", "2"))

                pending = []  # emitted-S'/exp units awaiting AV emission

                def flush_unit():
                    u = pending.pop(0)
                    for mmargs in u["av"]:
                        nc.tensor.matmul(**mmargs)
                    if u["fin"] is not None:
                        h, j, yps = u["fin"]
                        m, roff = divmod(h, 2)
                        roff *= 64
                        r = work.tile([1, 512], f32, tag="r", bufs=2, name="r")
                        nc.vector.reciprocal(r, yps[64:65, :])
                        rr = work.tile([64, 512], f32, tag="rr", bufs=2, name="rr")
                        nc.gpsimd.partition_broadcast(rr, r)
                        nc.vector.tensor_tensor(
                            yt_sb[roff:roff + 64, m, 512 * j:512 * j + 512],
                            yps[0:64, :], rr, mybir.AluOpType.mult,
                        )

                def attn_unit(h, j, i2, yps):
                    """Emit S' matmuls + exp for unit i2 of head-window (h, j)."""
                    m = h // 2
                    kd_h = kd_pad[:, h, :]
                    qd_h = qd_sb[:, m, :]
                    jwin = slice(512 * j, 512 * (j + 1))
                    nI = 4 * j + 4
                    sps = psums.tile([128, 1024], f32, tag="s", bufs=3, name="sps")
                    nomask = "nomask" in attn_mode
                    for u in (0, 1):
                        i = 2 * i2 + u
                        d = i - 4 * j  # >= 0 for diagonal-block chunks
                        if narrow and d > 0 and not nomask:
                            # diagonal chunk: columns < 128d are fully masked;
                            # compute only [128d, 512) plus a 128-wide
                            # triangular boundary mask (maskw block 0 is tri)
                            off = 128 * d
                            nc.tensor.matmul(
                                sps[:, 512 * u + off:512 * u + 512],
                                lhsT=kd_h[:, 128 * i:128 * i + 128],
                                rhs=qd_h[:, 512 * j + off:512 * j + 512],
                                start=True, stop=False,
                            )
                            nc.tensor.matmul(
                                sps[:, 512 * u + off:512 * u + off + 128],
                                lhsT=maskc_sb,                   # identity
                                rhs=maskw_sb[:, 0, 0:128],       # tri block
                                start=False, stop=True,
                            )
                            continue
                        nc.tensor.matmul(
                            sps[:, 512 * u:512 * u + 512],
                            lhsT=kd_h[:, 128 * i:128 * i + 128],
                            rhs=qd_h[:, jwin],
                            start=True, stop=(d < 0 or nomask),
                        )
                        if d >= 0 and not nomask:
                            nc.tensor.matmul(
                                sps[:, 512 * u:512 * u + 128 * (d + 1)],
                                lhsT=maskc_sb,                   # identity
                                rhs=maskw_sb[:, d, 0:128 * (d + 1)],
                                start=False, stop=True,
                            )
                    pt = work.tile([128, 1024], mm_dt, tag="p", bufs=6, name="pt")
                    f = exp_f if "expcopy" not in attn_mode else \
                        mybir.ActivationFunctionType.Copy
                    nc.scalar.activation(pt, sps, f, scale=0.125)
                    if attn_mode == "noav":
                        return
                    av = []
                    for u in (0, 1):
                        i = 2 * i2 + u
                        d = i - 4 * j
                        off = 128 * d if (narrow and d > 0) else 0
                        av.append(dict(out=yps[0:65, off:512],
                                       lhsT=v_sb[:, i, 65 * h:65 * h + 65],
                                       rhs=pt[:, 512 * u + off:512 * u + 512],
                                       start=(i == 0),
                                       stop=(i == nI - 1)))
                    pending.append(dict(
                        av=av, fin=(h, j, yps) if i2 == 2 * j + 1 else None))
                    while len(pending) > LAG:
                        flush_unit()

                # ---- static filler schedule ----
                # Keyed by the attention unit (j, h, i2) after which each
                # filler group is emitted. Ordering constraints:
                #   - qk m-group window w emitted before any S' of (heads of
                #     m, w)
                #   - V chunk tt emitted before AV of chunk tt
                #   - proj tt of window w emitted after window w's last
                #     normalize (which flushes LAG units into window w+1)
                QF = "qkv" in phases
                PF = "proj" in phases and attn_mode != "noav"
                K_, Q_, V_, P_ = "K", "Q", "V", "P"
                # at most ONE filler per unit slot: a clump of fillers puts
                # multiple allocations between consecutive sps tiles of the
                # shared ring, making the S' pipeline wait on drain latency
                sched_list = [
                    ((0, 0, 0), [(V_, 2)]),
                    ((0, 0, 1), [(V_, 3)]),
                    ((0, 1, 0), [(K_, 1, 0)]),
                    ((0, 1, 1), [(Q_, 1, 0)]),
                    ((0, 2, 0), [(K_, 0, 1)]),
                    ((0, 2, 1), [(Q_, 0, 1)]),
                    ((0, 3, 0), [(V_, 4)]),
                    ((0, 3, 1), [(V_, 5)]),
                    ((1, 0, 0), [(V_, 6)]),
                    ((1, 0, 1), [(V_, 7)]),
                    ((1, 0, 2), [(P_, 0)]),
                    ((1, 0, 3), [(P_, 1)]),
                    ((1, 1, 0), [(P_, 2)]),
                    ((1, 1, 1), [(P_, 3)]),
                    ((1, 1, 2), [(K_, 1, 1)]),
                    ((1, 1, 3), [(Q_, 1, 1)]),
                    ((1, 2, 0), [(V_, 8)]),
                    ((1, 2, 1), [(V_, 9)]),
                    ((1, 2, 2), [(K_, 0, 2)]),
                    ((1, 2, 3), [(Q_, 0, 2)]),
                    ((1, 3, 0), [(V_, 10)]),
                    ((1, 3, 1), [(V_, 11)]),
                    ((1, 3, 2), [(K_, 1, 2)]),
                    ((1, 3, 3), [(Q_, 1, 2)]),
                    ((2, 0, 2), [(P_, 4)]),
                    ((2, 0, 4), [(P_, 5)]),
                    ((2, 1, 0), [(P_, 6)]),
                    ((2, 1, 2), [(P_, 7)]),
                    ((2, 1, 4), [(V_, 12)]),
                    ((2, 2, 0), [(V_, 13)]),
                    ((2, 2, 2), [(K_, 0, 3)]),
                    ((2, 2, 4), [(Q_, 0, 3)]),
                    ((2, 3, 0), [(V_, 14)]),
                    ((2, 3, 2), [(V_, 15)]),
                    ((2, 3, 4), [(K_, 1, 3)]),
                    ((3, 0, 0), [(Q_, 1, 3)]),
                    ((3, 0, 2), [(P_, 8)]),
                    ((3, 0, 4), [(P_, 9)]),
                    ((3, 0, 6), [(P_, 10)]),
                    ((3, 1, 0), [(P_, 11)]),
                ]
                tail_fillers = [(P_, tt) for tt in range(12, 16)]

                def run_filler(f):
                    kind = f[0]
                    if kind == K_ and QF:
                        qk_group(1, f[1], f[2])
                    elif kind == Q_ and QF:
                        qk_group(0, f[1], f[2])
                    elif kind == V_ and QF:
                        v_group(f[1])
                    elif kind == P_ and PF:
                        proj_group(f[1])

                sched = {}
                for key, fl in sched_list:
                    sched.setdefault(key, []).extend(fl)

                if "attn" in phases:
                    # preamble: just enough qkv for (h=0, j=0) to start
                    # (V chunks 0,1 are consumed by the first flushed AVs)
                    if QF:
                        qk_group(1, 0, 0)
                        qk_group(0, 0, 0)
                        v_group(0)
                        v_group(1)
                    hlist = [0, 2, 0, 2] if "evenheads" in attn_mode else range(HG)
                    for j in range(NJ):
                        for h in hlist:
                            yps = None
                            if attn_mode != "noav":
                                yps = psum.tile([128, 512], f32, tag="y",
                                                bufs=2, name="yps")
                            for i2 in range(2 * j + 2):
                                attn_unit(h, j, i2, yps)
                                for f in sched.get((j, h, i2), ()):
                                    run_filler(f)
                    while pending:
                        flush_unit()
                    for f in tail_fillers:
                        run_filler(f)
                else:
                    # no attention: run the groups sequentially for bisection
                    if QF:
                        qk_group(1, 0, 0)
                        qk_group(0, 0, 0)
                        v_group(0)
                        v_group(1)
                    for _, fl in sched_list:
                        for f in fl:
                            run_filler(f)
                    for f in tail_fillers:
                        run_filler(f)

    nc.finalize()
    return nc


def make_in_maps(x, w_attn, b_attn, w_proj):
    x = np.asarray(x, dtype=np.float32)
    w_attn = np.asarray(w_attn, dtype=np.float32)
    b_attn = np.asarray(b_attn, dtype=np.float32)
    w_proj = np.asarray(w_proj, dtype=np.float32)

    ident = np.eye(128, dtype=np.float32)
    tri = np.where(np.triu(np.ones((128, 128), bool)), 0.0, -3000.0)
    blocks = [ident]
    for d in range(4):
        blk = np.zeros((128, 512), np.float32)
        blk[:, :128 * d] = -3000.0
        blk[:, 128 * d:128 * d + 128] = tri
        blocks.append(blk)
    mask_np = np.concatenate(blocks, axis=1).astype(BF16)  # [128, 128+2048]
    xts = [np.ascontiguousarray(x[b].T).astype(BF16) for b in range(B)]
    in_maps = []
    for core in range(NCORES):
        b, g = divmod(core, 4)
        cq = slice(0 * C + g * DQ, 0 * C + (g + 1) * DQ)
        ck = slice(1 * C + g * DQ, 1 * C + (g + 1) * DQ)
        cv = slice(2 * C + g * DQ, 2 * C + (g + 1) * DQ)
        bq = b_attn[cq]
        bk = b_attn[ck]
        bqk = np.stack([bq.reshape(2, 128), bk.reshape(2, 128)]).astype(np.float32)
        bv = np.broadcast_to(b_attn[cv], (128, DQ)).copy().astype(np.float32)
        in_maps.append({
            "xT": xts[b],
            "wq": np.ascontiguousarray(w_attn[:, cq]).astype(BF16),
            "wk": np.ascontiguousarray(w_attn[:, ck]).astype(BF16),
            "wv": np.ascontiguousarray(w_attn[:, cv]).astype(BF16),
            "wp": np.ascontiguousarray(w_proj[g * DQ:(g + 1) * DQ, :]).astype(BF16),
            "bqk": bqk,
            "bv": bv,
            "mask": mask_np,
        })
    return in_maps


def kernel(x, w_attn, b_attn, w_proj, b_proj):
    from concourse.bass_utils import run_bass_kernel_spmd

    if "nc" not in _NC_CACHE:
        _NC_CACHE["nc"] = build_nc()
    nc = _NC_CACHE["nc"]

    in_maps = make_in_maps(x, w_attn, b_attn, w_proj)
    res = run_bass_kernel_spmd(nc, in_maps, list(range(NCORES)))

    b_proj = np.asarray(b_proj, dtype=np.float32)
    out = np.zeros((B, T, C), np.float32)
    for core in range(NCORES):
        b = core // 4
        out[b] += res.results[core]["out"].astype(np.float32)
    out += b_proj[None, None, :]
    return out


# revision 17
# speedup vs baseline: 1.5969x; 1.1000x over previous
"""Causal self-attention (B=2, T=2048, C=1024, H=16, D=64) on 8 TRN2 NeuronCores.

Sharding (Megatron-style, per the hint): data-parallel over the batch (B=2)
and tensor-parallel over heads (16 heads -> 4 groups of 4). Core c handles
batch b = c // 4 and head group g = c % 4:
  - qkv:    computes x[b] @ w_attn[:, cols-of-its-4-heads]  (column split)
  - attn:   full causal attention for its 4 heads
  - proj:   y_heads @ w_proj[rows-of-its-4-heads]           (row split)
The 4 partial proj outputs per batch are summed on the host (+ b_proj).

Device layout notes:
  - All matmuls run in bf16 (inputs pre-cast/pre-transposed on host), fp32
    PSUM accumulation. The device output is bf16 (upcast on host).
  - Scores are computed transposed: S'[s, t] = (k_s . q_t)/8, so softmax sums
    over s (the partition dim) come for free out of the AV matmul by
    augmenting V with a ones column:  yT_aug = [V | 1]^T @ exp(S').
    Row 64 of yT_aug is the softmax denominator per t.
  - exp has no max-subtraction: logits are O(1) for this input distribution
    (|logit| < ~10), so fp32/bf16 exp is safe and the normalization cancels.

Scheduling notes (measured on HW, see exp_micro*.py):
  - Alternating the PE between 64- and 128-partition stationaries costs
    ~440ns per switch. K is therefore stored zero-PADDED per head
    (kd_pad[:, h]: head h's 64 d-rows at their natural partitions, zeros
    elsewhere) so S' runs as a uniform 128-partition matmul; the zero rows
    null out the other head's q rows in the full-width rhs.
  - The attention S'->exp->AV chain leaves PE idle gaps (ACT exp ~1.2us per
    [128,1024] tile), so qkv and proj matmul groups are woven INTO the
    attention stream as filler work at statically chosen points.
  - Diagonal-block chunks are narrowed to their causally-valid column range
    (plus a 128-wide triangular boundary mask folded into the PE
    accumulation group via an identity matmul).
"""

import os
import sys

sys.path.insert(0, "/opt/trn_rl_repo")

import numpy as np
import ml_dtypes

BF16 = ml_dtypes.bfloat16

B, T, C, H, D = 2, 2048, 1024, 16, 64
NCORES = 8
HG = 4          # heads per core
DQ = HG * D     # 256 qkv cols per core
CCH = C // 128  # 8 contraction chunks
NT = T // 128   # 16 token chunks of 128
NJ = T // 512   # 4 token tiles of 512

_NC_CACHE = {}


def build_nc(mm_dtype_name="bfloat16", loop=0, phases=("qkv", "attn", "proj"),
             attn_mode="full", narrow=True, widemask=False):
    """loop=0: straight-line (graded path). loop=K>0: wrap the body in a
    device-side For_i repeat-K loop (timing builds only). phases: subset for
    bisection timing builds."""
    import contextlib
    import concourse.bacc as bacc
    import concourse.tile as tile
    from concourse import mybir

    mm_dt = getattr(mybir.dt, mm_dtype_name)
    f32 = mybir.dt.float32

    nc = bacc.Bacc("TRN2", target_bir_lowering=False, debug=False,
                   num_devices=NCORES)

    xT = nc.dram_tensor("xT", [C, T], mm_dt, kind="ExternalInput")
    wq = nc.dram_tensor("wq", [C, DQ], mm_dt, kind="ExternalInput")
    wk = nc.dram_tensor("wk", [C, DQ], mm_dt, kind="ExternalInput")
    wv = nc.dram_tensor("wv", [C, DQ], mm_dt, kind="ExternalInput")
    wp = nc.dram_tensor("wp", [DQ, C], mm_dt, kind="ExternalInput")
    bqk = nc.dram_tensor("bqk", [2, 2, 128], f32, kind="ExternalInput")  # [q/k, chunk, col]
    bv = nc.dram_tensor("bv", [128, DQ], f32, kind="ExternalInput")      # replicated
    mask = nc.dram_tensor("mask", [128, 128 + 4 * 512], mm_dt, kind="ExternalInput")
    out = nc.dram_tensor("out", [T, C], mm_dt, kind="ExternalOutput")

    with tile.TileContext(nc) as tc:
        with (
            tc.tile_pool(name="const", bufs=1) as const,
            tc.tile_pool(name="acts", bufs=1) as acts,
            tc.tile_pool(name="work", bufs=4) as work,
            tc.tile_pool(name="ostage", bufs=3) as ostage,
            tc.tile_pool(name="psum", bufs=1, space="PSUM") as psum,
            tc.tile_pool(name="psums", bufs=1, space="PSUM") as psums,
        ):
            # ---- activations (allocated once; persistent across loop) ----
            qd_sb = acts.tile([128, 2, T], mm_dt)   # [dcol, chunk, t]
            kd_pad = acts.tile([128, HG, T], mm_dt)  # per-head, zero-padded
            v_sb = acts.tile([128, NT, HG * 65], mm_dt)  # per s-chunk: 4x [V_h | 1]
            yt_sb = acts.tile([128, 2, T], mm_dt)

            # one-time init (never overwritten by the loop body): the zero
            # padding rows of kd_pad and the ones columns of v_sb
            nc.vector.memset(kd_pad, 0.0)
            ones_view = v_sb.rearrange("p s (h e) -> p s h e", e=65)[:, :, :, 64:65]
            nc.vector.memset(ones_view, 1.0)
            # bisection timing builds: initialize tensors a skipped phase
            # would have produced
            if "qkv" not in phases:
                nc.vector.memset(qd_sb, 0.5)
                for h in range(HG):
                    roff = 64 * (h % 2)
                    nc.vector.memset(kd_pad[roff:roff + 64, h, :], 0.5)
                nc.vector.memset(
                    v_sb.rearrange("p s (h e) -> p s h e", e=65)[:, :, :, 0:64],
                    0.5)
            if "attn" not in phases or attn_mode == "noav":
                nc.vector.memset(yt_sb, 0.5)

            with (
                tc.For_i(0, loop, 1,
                         hint_engines=(mybir.EngineType.PE,
                                       mybir.EngineType.Activation,
                                       mybir.EngineType.DVE,
                                       mybir.EngineType.SP,
                                       mybir.EngineType.Pool))
                if loop else contextlib.nullcontext()
            ):
                # ---- constants / weights ----
                bqk_sb = const.tile([128, 2, 2, 1], f32)  # [col, q/k, chunk, 1]
                nc.sync.dma_start(out=bqk_sb,
                                  in_=bqk.rearrange("a m p -> p a m")[:, :, :, None])
                bv_sb = const.tile([128, DQ], f32)
                nc.sync.dma_start(out=bv_sb, in_=bv[:, :])
                # mask holds [ident(128) | 4 x 512 wide additive diag-block masks]
                maskc_sb = const.tile([128, 128], mm_dt)
                nc.sync.dma_start(out=maskc_sb, in_=mask[:, 0:128])
                maskw_sb = const.tile([128, 4, 512], mm_dt)
                nc.sync.dma_start(out=maskw_sb,
                                  in_=mask[:, 128:].rearrange("p (a n) -> p a n", a=4))
                wk_sb = const.tile([128, CCH, DQ], mm_dt)
                nc.sync.dma_start(out=wk_sb, in_=wk.rearrange("(c p) m -> p c m", p=128))
                wq_sb = const.tile([128, CCH, DQ], mm_dt)
                nc.sync.dma_start(out=wq_sb, in_=wq.rearrange("(c p) m -> p c m", p=128))
                # xT loaded in 4 column windows so window-0 qkv starts early
                xT_sb = const.tile([128, CCH, T], mm_dt)
                xTr = xT.rearrange("(c p) t -> p c t", p=128)
                for jw in range(NJ):
                    nc.sync.dma_start(out=xT_sb[:, :, 512 * jw:512 * jw + 512],
                                      in_=xTr[:, :, 512 * jw:512 * jw + 512])
                wv_sb = const.tile([128, CCH, DQ], mm_dt)
                nc.sync.dma_start(out=wv_sb, in_=wv.rearrange("(c p) m -> p c m", p=128))
                wp_sb = const.tile([128, 2, C], mm_dt)
                nc.sync.dma_start(out=wp_sb, in_=wp.rearrange("(k p) n -> p k n", p=128))

                # ---- PE work groups (attention units + filler groups) ----
                # Filler groups allocate psum from the shared tag-"s" ring so
                # PSUM stays within 8 banks.
                def qk_group(qki, m, j):
                    ps = psums.tile([128, 1024], f32, tag="s", bufs=3, name="ps_f")
                    wsb = wq_sb if qki == 0 else wk_sb
                    for c in range(CCH):
                        nc.tensor.matmul(
                            ps[:, 0:512],
                            lhsT=wsb[:, c, 128 * m:128 * m + 128],
                            rhs=xT_sb[:, c, 512 * j:512 * j + 512],
                            start=(c == 0), stop=(c == CCH - 1),
                        )
                    jw = slice(512 * j, 512 * j + 512)
                    if qki == 0:
                        nc.vector.tensor_scalar_add(
                            qd_sb[:, m, jw], ps[:, 0:512], bqk_sb[:, 0, m, :])
                    else:
                        # split into the two per-head zero-padded slots
                        nc.vector.tensor_scalar_add(
                            kd_pad[0:64, 2 * m, jw], ps[0:64, 0:512],
                            bqk_sb[0:64, 1, m, :])
                        nc.vector.tensor_scalar_add(
                            kd_pad[64:128, 2 * m + 1, jw], ps[64:128, 0:512],
                            bqk_sb[64:128, 1, m, :])

                def v_group(tt):
                    # V in s-major [t, vcol]; out tile = xT_chunk(t)^T @ Wv_chunk
                    ps = psums.tile([128, 1024], f32, tag="s", bufs=3, name="ps_f")
                    for c in range(CCH):
                        nc.tensor.matmul(
                            ps[:, 0:DQ],
                            lhsT=xT_sb[:, c, 128 * tt:128 * tt + 128],
                            rhs=wv_sb[:, c, :],
                            start=(c == 0), stop=(c == CCH - 1),
                        )
                    nc.vector.tensor_tensor(
                        v_sb.rearrange("p s (h e) -> p s h e", e=65)[:, tt, :, 0:64],
                        ps[:, 0:DQ].rearrange("p (h d) -> p h d", d=64),
                        bv_sb.rearrange("p (h d) -> p h d", d=64),
                        mybir.AluOpType.add,
                    )

                def proj_group(tt):
                    ps = psums.tile([128, 1024], f32, tag="s", bufs=3, name="ps_o")
                    for n2 in range(2):
                        for kc in range(2):
                            nc.tensor.matmul(
                                ps[:, 512 * n2:512 * n2 + 512],
                                lhsT=yt_sb[:, kc, 128 * tt:128 * tt + 128],
                                rhs=wp_sb[:, kc, 512 * n2:512 * n2 + 512],
                                start=(kc == 0), stop=(kc == 1),
                            )
                    os_sb = ostage.tile([128, C], mm_dt, tag="osb", name="os_sb")
                    # drain on ACT: keeps the shared psum ring's release off
                    # the (busier) DVE queue
                    nc.scalar.copy(os_sb, ps)
                    nc.sync.dma_start(out=out[128 * tt:128 * tt + 128, :],
                                      in_=os_sb)

                # ---- attention units, software-pipelined ----
                exp_f = mybir.ActivationFunctionType.Exp
                LAG = int(os.environ.get("ATTN_LAG", "2"))

                pending = []  # emitted-S'/exp units awaiting AV emission

                def flush_unit():
                    u = pending.pop(0)
                    for mmargs in u["av"]:
                        nc.tensor.matmul(**mmargs)
                    if u["fin"] is not None:
                        h, j, yps = u["fin"]
                        m, roff = divmod(h, 2)
                        roff *= 64
                        r = work.tile([1, 512], f32, tag="r", bufs=2, name="r")
                        nc.vector.reciprocal(r, yps[64:65, :])
                        rr = work.tile([64, 512], f32, tag="rr", bufs=2, name="rr")
                        nc.gpsimd.partition_broadcast(rr, r)
                        nc.vector.tensor_tensor(
                            yt_sb[roff:roff + 64, m, 512 * j:512 * j + 512],
                            yps[0:64, :], rr, mybir.AluOpType.mult,
                        )

                def attn_unit(h, j, i2, yps):
                    """Emit S' matmuls + exp for unit i2 of head-window (h, j)."""
                    m = h // 2
                    kd_h = kd_pad[:, h, :]
                    qd_h = qd_sb[:, m, :]
                    jwin = slice(512 * j, 512 * (j + 1))
                    nI = 4 * j + 4
                    sps = psums.tile([128, 1024], f32, tag="s", bufs=3, name="sps")
                    nomask = "nomask" in attn_mode
                    for u in (0, 1):
                        i = 2 * i2 + u
                        d = i - 4 * j  # >= 0 for diagonal-block chunks
                        if narrow and d > 0 and not nomask:
                            # diagonal chunk: columns < 128d are fully masked;
                            # compute only [128d, 512) plus a 128-wide
                            # triangular boundary mask (maskw block 0 is tri)
                            off = 128 * d
                            nc.tensor.matmul(
                                sps[:, 512 * u + off:512 * u + 512],
                                lhsT=kd_h[:, 128 * i:128 * i + 128],
                                rhs=qd_h[:, 512 * j + off:512 * j + 512],
                                start=True, stop=False,
                            )
                            nc.tensor.matmul(
                                sps[:, 512 * u + off:512 * u + off + 128],
                                lhsT=maskc_sb,                   # identity
                                rhs=maskw_sb[:, 0, 0:128],       # tri block
                                start=False, stop=True,
                            )
                            continue
                        nc.tensor.matmul(
                            sps[:, 512 * u:512 * u + 512],
                            lhsT=kd_h[:, 128 * i:128 * i + 128],
                            rhs=qd_h[:, jwin],
                            start=True, stop=(d < 0 or nomask),
                        )
                        if d >= 0 and not nomask:
                            # widemask: full 512-wide add (mask block is zero
                            # past the triangle) keeps every PE matmul at a
                            # uniform 512 moving width
                            mw = 512 if widemask else 128 * (d + 1)
                            nc.tensor.matmul(
                                sps[:, 512 * u:512 * u + mw],
                                lhsT=maskc_sb,                   # identity
                                rhs=maskw_sb[:, d, 0:mw],
                                start=False, stop=True,
                            )
                    pt = work.tile([128, 1024], mm_dt, tag="p", bufs=6, name="pt")
                    f = exp_f if "expcopy" not in attn_mode else \
                        mybir.ActivationFunctionType.Copy
                    nc.scalar.activation(pt, sps, f, scale=0.125)
                    if attn_mode == "noav":
                        return
                    av = []
                    for u in (0, 1):
                        i = 2 * i2 + u
                        d = i - 4 * j
                        off = 128 * d if (narrow and d > 0) else 0
                        av.append(dict(out=yps[0:65, off:512],
                                       lhsT=v_sb[:, i, 65 * h:65 * h + 65],
                                       rhs=pt[:, 512 * u + off:512 * u + 512],
                                       start=(i == 0),
                                       stop=(i == nI - 1)))
                    pending.append(dict(
                        av=av, fin=(h, j, yps) if i2 == 2 * j + 1 else None))
                    while len(pending) > LAG:
                        flush_unit()

                # ---- static filler schedule ----
                # Keyed by the attention unit (j, h, i2) after which each
                # filler group is emitted. Ordering constraints:
                #   - qk m-group window w emitted before any S' of (heads of
                #     m, w)
                #   - V chunk tt emitted before AV of chunk tt
                #   - proj tt of window w emitted after window w's last
                #     normalize (which flushes LAG units into window w+1)
                QF = "qkv" in phases
                PF = "proj" in phases and attn_mode != "noav"
                K_, Q_, V_, P_ = "K", "Q", "V", "P"
                # at most ONE filler per unit slot: a clump of fillers puts
                # multiple allocations between consecutive sps tiles of the
                # shared ring, making the S' pipeline wait on drain latency
                sched_list = [
                    ((0, 0, 0), [(V_, 2)]),
                    ((0, 0, 1), [(V_, 3)]),
                    ((0, 1, 0), [(K_, 1, 0)]),
                    ((0, 1, 1), [(Q_, 1, 0)]),
                    ((0, 2, 0), [(K_, 0, 1)]),
                    ((0, 2, 1), [(Q_, 0, 1)]),
                    ((0, 3, 0), [(V_, 4)]),
                    ((0, 3, 1), [(V_, 5)]),
                    ((1, 0, 0), [(V_, 6)]),
                    ((1, 0, 1), [(V_, 7)]),
                    ((1, 0, 2), [(P_, 0)]),
                    ((1, 0, 3), [(P_, 1)]),
                    ((1, 1, 0), [(P_, 2)]),
                    ((1, 1, 1), [(P_, 3)]),
                    ((1, 1, 2), [(K_, 1, 1)]),
                    ((1, 1, 3), [(Q_, 1, 1)]),
                    ((1, 2, 0), [(V_, 8)]),
                    ((1, 2, 1), [(V_, 9)]),
                    ((1, 2, 2), [(K_, 0, 2)]),
                    ((1, 2, 3), [(Q_, 0, 2)]),
                    ((1, 3, 0), [(V_, 10)]),
                    ((1, 3, 1), [(V_, 11)]),
                    ((1, 3, 2), [(K_, 1, 2)]),
                    ((1, 3, 3), [(Q_, 1, 2)]),
                    ((2, 0, 2), [(P_, 4)]),
                    ((2, 0, 4), [(P_, 5)]),
                    ((2, 1, 0), [(P_, 6)]),
                    ((2, 1, 2), [(P_, 7)]),
                    ((2, 1, 4), [(V_, 12)]),
                    ((2, 2, 0), [(V_, 13)]),
                    ((2, 2, 2), [(K_, 0, 3)]),
                    ((2, 2, 4), [(Q_, 0, 3)]),
                    ((2, 3, 0), [(V_, 14)]),
                    ((2, 3, 2), [(V_, 15)]),
                    ((2, 3, 4), [(K_, 1, 3)]),
                    ((3, 0, 0), [(Q_, 1, 3)]),
                    ((3, 0, 2), [(P_, 8)]),
                    ((3, 0, 4), [(P_, 9)]),
                    ((3, 0, 6), [(P_, 10)]),
                    ((3, 1, 0), [(P_, 11)]),
                ]
                tail_fillers = [(P_, tt) for tt in range(12, 16)]

                def run_filler(f):
                    kind = f[0]
                    if kind == K_ and QF:
                        qk_group(1, f[1], f[2])
                    elif kind == Q_ and QF:
                        qk_group(0, f[1], f[2])
                    elif kind == V_ and QF:
                        v_group(f[1])
                    elif kind == P_ and PF:
                        proj_group(f[1])

                sched = {}
                for key, fl in sched_list:
                    sched.setdefault(key, []).extend(fl)

                if "attn" in phases:
                    # preamble: just enough qkv for (h=0, j=0) to start
                    # (V chunks 0,1 are consumed by the first flushed AVs)
                    if QF:
                        qk_group(1, 0, 0)
                        qk_group(0, 0, 0)
                        v_group(0)
                        v_group(1)
                    hlist = [0, 2, 0, 2] if "evenheads" in attn_mode else range(HG)
                    for j in range(NJ):
                        for h in hlist:
                            yps = None
                            if attn_mode != "noav":
                                yps = psum.tile([128, 512], f32, tag="y",
                                                bufs=2, name="yps")
                            for i2 in range(2 * j + 2):
                                attn_unit(h, j, i2, yps)
                                for f in sched.get((j, h, i2), ()):
                                    run_filler(f)
                    while pending:
                        flush_unit()
                    for f in tail_fillers:
                        run_filler(f)
                else:
                    # no attention: run the groups sequentially for bisection
                    if QF:
                        qk_group(1, 0, 0)
                        qk_group(0, 0, 0)
                        v_group(0)
                        v_group(1)
                    for _, fl in sched_list:
                        for f in fl:
                            run_filler(f)
                    for f in tail_fillers:
                        run_filler(f)

    nc.finalize()
    return nc


def make_in_maps(x, w_attn, b_attn, w_proj):
    x = np.asarray(x, dtype=np.float32)
    w_attn = np.asarray(w_attn, dtype=np.float32)
    b_attn = np.asarray(b_attn, dtype=np.float32)
    w_proj = np.asarray(w_proj, dtype=np.float32)

    ident = np.eye(128, dtype=np.float32)
    tri = np.where(np.triu(np.ones((128, 128), bool)), 0.0, -3000.0)
    blocks = [ident]
    for d in range(4):
        blk = np.zeros((128, 512), np.float32)
        blk[:, :128 * d] = -3000.0
        blk[:, 128 * d:128 * d + 128] = tri
        blocks.append(blk)
    mask_np = np.concatenate(blocks, axis=1).astype(BF16)  # [128, 128+2048]
    xts = [np.ascontiguousarray(x[b].T).astype(BF16) for b in range(B)]
    in_maps = []
    for core in range(NCORES):
        b, g = divmod(core, 4)
        cq = slice(0 * C + g * DQ, 0 * C + (g + 1) * DQ)
        ck = slice(1 * C + g * DQ, 1 * C + (g + 1) * DQ)
        cv = slice(2 * C + g * DQ, 2 * C + (g + 1) * DQ)
        bq = b_attn[cq]
        bk = b_attn[ck]
        bqk = np.stack([bq.reshape(2, 128), bk.reshape(2, 128)]).astype(np.float32)
        bv = np.broadcast_to(b_attn[cv], (128, DQ)).copy().astype(np.float32)
        in_maps.append({
            "xT": xts[b],
            "wq": np.ascontiguousarray(w_attn[:, cq]).astype(BF16),
            "wk": np.ascontiguousarray(w_attn[:, ck]).astype(BF16),
            "wv": np.ascontiguousarray(w_attn[:, cv]).astype(BF16),
            "wp": np.ascontiguousarray(w_proj[g * DQ:(g + 1) * DQ, :]).astype(BF16),
            "bqk": bqk,
            "bv": bv,
            "mask": mask_np,
        })
    return in_maps


def kernel(x, w_attn, b_attn, w_proj, b_proj):
    from concourse.bass_utils import run_bass_kernel_spmd

    if "nc" not in _NC_CACHE:
        _NC_CACHE["nc"] = build_nc()
    nc = _NC_CACHE["nc"]

    in_maps = make_in_maps(x, w_attn, b_attn, w_proj)
    res = run_bass_kernel_spmd(nc, in_maps, list(range(NCORES)))

    b_proj = np.asarray(b_proj, dtype=np.float32)
    out = np.zeros((B, T, C), np.float32)
    for core in range(NCORES):
        b = core // 4
        out[b] += res.results[core]["out"].astype(np.float32)
    out += b_proj[None, None, :]
    return out
